# revision 1
# baseline (speedup 1.0000x reference)
"""Cross-attention kernel for Trainium2 (8 NeuronCores, batch-parallel).

Reference computation (per batch element b):
    q = x @ Wq + bq            # [T, E]
    k = y @ Wk + bk            # [S, E]
    v = y @ Wv + bv            # [S, E]
    per head h (D=80): scores = q_h @ k_h.T / sqrt(D); A = softmax(scores)
    attn = concat_h(A @ v_h)   # [T, E]
    out = attn @ Wo + bo       # [T, E]

Sharding: batch (8) across the 8 cores, one batch element per core.

Design (vs the f32 baseline; cost-model time 210us -> 134us):
  - bf16 for every DMA'd tensor (x, y, weights, output) and the matmul
    streams; PSUM accumulation stays f32, softmax sums stay f32. Halves
    HBM traffic and SBUF pressure (rel tolerance is 2e-2, measured HW
    error ~4.4e-3).
  - softmax denominators via gpsimd partition_all_reduce +
    reciprocal_approx_fast + normalize multiplies alternating DVE/Pool:
    removes the baseline's 8 per-chunk sum-of-exp matmuls and its
    latency-heavy DMA+partition_broadcast chain.
  - heads whose 80 features straddle a 128-partition tile boundary
    (h1/h3/h4/h6) get their k and q repacked into dedicated 128-row
    tiles (SBUF-SBUF DMA, one chunk ahead), collapsing their two score
    fragments into ONE matmul: scores cost 8 MMs/chunk instead of 12.
  - one big DMA per chunk for x loads and output stores ([128, 5, tw]
    rearranged views) instead of 5 small ones (HWDGE fixed cost paid
    once); weight loads split by column block so Vproj/Kproj/Qproj
    start as soon as their slice lands; wv/wk/wo loads and the output
    stores issue from the ACT-engine HWDGE ring (qActDynamicHW) so the
    two physical descriptor-generation rings run in parallel.
  - software pipelining: per chunk the PE stream is scores(i) ->
    Oproj(i-1) -> Qproj(i+1) -> AV(i), so the softmax normalization
    chain of chunk i hides under ~10us of projection matmuls.
  - PE warmup matmuls during the initial weight-DMA fill so the
    p-state/HAM ramp completes before real work arrives; narrow first/
    last chunks shrink pipeline fill and drain; the last chunk
    interleaves its O-projection j-groups with the AV tiles.

On-chip layout is feature-major (x and the output are transposed on the
host): xt = x[b].T [E, T]; q' = Wq.T@X' [E, T]; per-head zero-padded
k/v staging tiles (80-dim heads vs 128-partition tiles).
"""

import numpy as np

import concourse.bass as bass
import concourse.bass_isa as bass_isa
import concourse.mybir as mybir
import concourse.tile as tile
from concourse import bacc
from concourse.bass_utils import run_bass_kernel_spmd

F32 = mybir.dt.float32
F32R = mybir.dt.float32r
BF16 = mybir.dt.bfloat16
AF = mybir.ActivationFunctionType

B, T, E, CR, H, D, S = 8, 4096, 640, 768, 8, 80, 77
TC = 512
# narrow edge chunks: chunk 0 starts sooner (less DMA before the pipeline
# start) and the last chunk's O-projection + store can't overlap anything.
CHUNKS = ([(0, 256)] + [(256 + 512 * i, 512) for i in range(7)]
          + [(3840, 256)])
NT = len(CHUNKS)
EJ = E // 128            # 5 e-tiles
CJ = CR // 128           # 6 cross-dim tiles
SCALE = float(1.0 / np.sqrt(D))
NWARM = 77               # PE warmup matmuls during DMA fill (75 is the
                         # sim optimum but sits one MM from a 1.3us
                         # phase-alignment cliff; 77 keeps margin)
AT_ACT_N = 4             # at-copies j < AT_ACT_N go to ACT, rest to DVE


def _frags():
    fr = []
    for h in range(H):
        e0, e1 = D * h, D * h + D
        for j in range(e0 // 128, (e1 - 1) // 128 + 1):
            p0, p1 = max(0, e0 - 128 * j), min(128, e1 - 128 * j)
            fr.append((h, j, p0, p1))
    return fr


FRAGS = _frags()         # 12 fragments
NF = len(FRAGS)
# heads whose 80 features straddle a 128-partition tile boundary (two
# fragments); their scores run as ONE matmul against a repacked q tile
STRADDLE = []            # (h, fiA, fiB, j, p0, p1): fragA=(j,p0,128), fragB=(j+1,0,p1)
for h in range(H):
    frs = [(fi, f) for fi, f in enumerate(FRAGS) if f[0] == h]
    if len(frs) == 2:
        (fiA, (_, jA, p0A, _)), (fiB, (_, jB, _, p1B)) = frs
        STRADDLE.append((h, fiA, fiB, jA, p0A, p1B))
SINGLE = [(h, [fi for fi, f in enumerate(FRAGS) if f[0] == h][0])
          for h in range(H)
          if len([fi for fi, f in enumerate(FRAGS) if f[0] == h]) == 1]


def _emit(nc, tc, dr):
    import contextlib

    ctx = contextlib.ExitStack()
    with ctx:
        cpool = ctx.enter_context(tc.tile_pool(name="const", bufs=1))
        pq = ctx.enter_context(tc.tile_pool(name="pq", bufs=2, space="PSUM"))
        psc = ctx.enter_context(tc.tile_pool(name="psc", bufs=2, space="PSUM"))
        pav = ctx.enter_context(tc.tile_pool(name="pav", bufs=2, space="PSUM"))
        pop = ctx.enter_context(tc.tile_pool(name="pop", bufs=2, space="PSUM"))
        xpool = ctx.enter_context(tc.tile_pool(name="xpool", bufs=2))
        qpool = ctx.enter_context(tc.tile_pool(name="qpool", bufs=2))
        apool = ctx.enter_context(tc.tile_pool(name="apool", bufs=2))
        arpool = ctx.enter_context(tc.tile_pool(name="arpool", bufs=3))
        atpool = ctx.enter_context(tc.tile_pool(name="atpool", bufs=2))
        opool = ctx.enter_context(tc.tile_pool(name="opool", bufs=2))

        xtr = dr["xt"].rearrange("(b p) c -> p b c", p=128)
        otr = dr["ot"].rearrange("(b p) c -> p b c", p=128)

        # ---- PE warmup: keep the array busy through the DMA fill ----
        wtile = cpool.tile([128, 128], BF16, tag="warm", name="warm")
        nc.vector.memset(wtile[:], 0.0)
        wps = pop.tile([128, 128], F32, tag="op", name="warmps")
        for _ in range(NWARM):
            nc.tensor.matmul(wps[:], wtile[:], wtile[:], start=True, stop=True)

        # ---- weight/const loads (order = DMA queue order). The big
        # weights are split by column block so dependent PE groups can
        # start as soon as their slice lands.
        def loadw(name, src2, nblk, cols, c0=0, c1=None, dt=BF16,
                  eng=None):
            c1 = cols if c1 is None else c1
            t = cpool.tile([128, nblk, c1 - c0], dt, tag=name, name=name)
            (eng or nc.sync).dma_start(
                t[:], src2.rearrange("(b p) c -> p b c", p=128)[:, :, c0:c1])
            return t

        ytp_t = loadw("yt", dr["yt"], CJ, S)
        consts2 = cpool.tile([1, E + S], BF16, tag="consts2", name="consts2")
        nc.sync.dma_start(consts2[:], dr["consts2"])
        consts = cpool.tile([128, 27], F32, tag="consts", name="consts")
        nc.sync.dma_start(consts[:], dr["consts"])
        # wv via the ACT-engine HWDGE ring (qActDynamicHW on HW): its
        # descriptor generation runs parallel to the SP-ring loads
        wv_a = loadw("wva", dr["wv"], CJ, E, 0, 512, eng=nc.scalar)
        wv_b = loadw("wvb", dr["wv"], CJ, E, 512, E, eng=nc.scalar)
        wk_a = loadw("wka", dr["wk"], CJ, E, 0, 256, eng=nc.scalar)
        wk_b = loadw("wkb", dr["wk"], CJ, E, 256, E, eng=nc.scalar)
        wq_a = loadw("wqa", dr["wq"], EJ, E, 0, 256)
        xp0 = xpool.tile([128, EJ, TC], BF16, tag="xp", name="xp0")
        nc.sync.dma_start(xp0[0:128, 0:EJ, 0:CHUNKS[0][1]],
                          xtr[:, :, CHUNKS[0][0]:CHUNKS[0][0] + CHUNKS[0][1]])
        wq_b = loadw("wqb", dr["wq"], EJ, E, 256, E)
        xp1 = xpool.tile([128, EJ, TC], BF16, tag="xp", name="xp1")
        nc.sync.dma_start(xp1[0:128, 0:EJ, 0:CHUNKS[1][1]],
                          xtr[:, :, CHUNKS[1][0]:CHUNKS[1][0] + CHUNKS[1][1]])
        # qpk(0) DMAs, wo + remaining x chunks are issued further down

        def wq_col(j, c):
            return (wq_a[:, c, 128 * j:128 * (j + 1)] if j < 2
                    else wq_b[:, c, 128 * (j - 2):128 * (j - 1)])

        def wk_col(j, c):
            return (wk_a[:, c, 128 * j:128 * (j + 1)] if j < 2
                    else wk_b[:, c, 128 * (j - 2):128 * (j - 1)])

        bqt = consts[:, 0:EJ]
        bkt = consts[:, EJ:2 * EJ]
        bot = consts[:, 2 * EJ:3 * EJ]
        kmask = consts[:, 3 * EJ:3 * EJ + NF]
        bvr = consts2[:, 0:E]
        ones77 = consts2[:, E:E + S]

        # ---- V projection -> vb fragments (zero-padded) ----
        # vb holds, per fragment, a [S, 128] slab that is zero outside the
        # head's partition range; built by memset + free-dim-offset copies.
        vb = cpool.tile([S, NF * 128], BF16, tag="vb", name="vb")
        nc.gpsimd.memset(vb[:], 0.0)

        def emit_vproj():
            for (n0, n1), wv_t in (((0, 512), wv_a), ((512, E), wv_b)):
                vp = psc.tile([S, n1 - n0], F32, tag="sc")
                for c in range(CJ):
                    nc.tensor.matmul(vp[:], ytp_t[:, c, :],
                                     wv_t[:, c, :],
                                     start=(c == 0), stop=False)
                nc.tensor.matmul(vp[:], ones77[:], bvr[:, n0:n1],
                                 start=False, stop=True)
                for fi, (h, j, p0, p1) in enumerate(FRAGS):
                    c0 = 128 * j
                    if not (n0 <= c0 and c0 + 128 <= n1):
                        continue
                    # on ACT (idle during setup): keeps DVE clear for the
                    # kstage ops that gate Kproj's psum recycling
                    nc.scalar.activation(
                        vb[:, 128 * fi + p0:128 * fi + p1],
                        vp[:, c0 - n0 + p0:c0 - n0 + p1], AF.Copy)

        # ---- K projection -> zero-masked per-fragment staging tiles ----
        kstage = [cpool.tile([128, S], BF16, tag=f"ks{fi}", name=f"ks{fi}")
                  for fi in range(NF)]

        def emit_kproj():
            for j in range(EJ):
                # alternate psum pools (pav is idle during setup) so the
                # kstage DVE ops don't gate the bank recycling
                kpool = pq if j % 2 == 0 else pav
                kp = kpool.tile([128, S], F32, tag="qp" if j % 2 == 0
                                else "av")
                for c in range(CJ):
                    nc.tensor.matmul(kp[:], wk_col(j, c),
                                     ytp_t[:, c, :], start=(c == 0),
                                     stop=(c == CJ - 1))
                for fi, (h, jj, p0, p1) in enumerate(FRAGS):
                    if jj != j:
                        continue
                    nc.vector.tensor_scalar(kstage[fi][:], kp[:],
                                            bkt[:, j:j + 1],
                                            kmask[:, fi:fi + 1],
                                            mybir.AluOpType.add,
                                            mybir.AluOpType.mult)

        def emit_qproj(xp, tw, qpools=None):
            qs = []
            for j in range(EJ):
                pool, ptag = (qpools[j] if qpools
                              else (pq, "qp"))
                qp = pool.tile([128, TC], F32, tag=ptag, name=f"qp{j}")
                for c in range(EJ):
                    nc.tensor.matmul(qp[0:128, 0:tw],
                                     wq_col(j, c),
                                     xp[0:128, c, 0:tw],
                                     start=(c == 0), stop=(c == EJ - 1))
                q = qpool.tile([128, TC], BF16, tag=f"q{j}", name=f"q{j}")
                nc.vector.tensor_scalar_add(q[0:128, 0:tw], qp[0:128, 0:tw],
                                            bqt[:, j:j + 1])
                qs.append(q)
            return qs

        # ---- packed k/q staging for straddling heads: their two score
        # fragments collapse to ONE matmul against a q tile whose rows
        # 0:80 are the head's features (repacked via SBUF-SBUF DMA).
        kpk = {}
        qpk = {}
        for (h, fiA, fiB, j, p0, p1) in STRADDLE:
            kpk[h] = cpool.tile([128, S], BF16, tag=f"kpk{h}", name=f"kpk{h}")
            qpk[h] = cpool.tile([128, TC], BF16, tag=f"qpk{h}",
                                name=f"qpk{h}")
            nc.gpsimd.memset(kpk[h][:], 0.0)
            nc.gpsimd.memset(qpk[h][:], 0.0)

        def emit_kpk():
            for (h, fiA, fiB, j, p0, p1) in STRADDLE:
                nc.sync.dma_start(kpk[h][0:128 - p0, :],
                                  kstage[fiA][p0:128, :])
                nc.sync.dma_start(kpk[h][128 - p0:D, :],
                                  kstage[fiB][0:p1, :])

        def emit_qpk(qs, tw):
            for (h, fiA, fiB, j, p0, p1) in STRADDLE:
                nc.sync.dma_start(qpk[h][0:128 - p0, 0:tw],
                                  qs[j][p0:128, 0:tw])
                nc.sync.dma_start(qpk[h][128 - p0:D, 0:tw],
                                  qs[j + 1][0:p1, 0:tw])

        def _softmax_tail(h, sc, tw, aps):
            """exp -> allreduce(sumexp) -> approx-recip -> normalize.
            The normalize multiplies alternate DVE/Pool to spread the
            elementwise load (DVE is near-saturated in steady state)."""
            a = apool.tile([S, TC], BF16, tag=f"a{h}", name=f"a{h}")
            nc.scalar.activation(a[0:S, 0:tw], sc[0:S, 0:tw], AF.Exp,
                                 scale=SCALE)
            ar = arpool.tile([S, TC], F32, tag="ar")
            nc.gpsimd.partition_all_reduce(
                ar[0:S, 0:tw], a[0:S, 0:tw], channels=S,
                reduce_op=bass_isa.ReduceOp.add)
            nc.vector.reciprocal_approx_fast(ar[0:S, 0:tw], ar[0:S, 0:tw])
            eng = nc.vector if h % 2 == 0 else nc.gpsimd
            eng.tensor_mul(a[0:S, 0:tw], a[0:S, 0:tw], ar[0:S, 0:tw])
            aps[h] = a

        def emit_scores_all(qs, tw, aps):
            """fragment-accumulation scores for all heads (chunk 0 only,
            before the packed q staging pipeline is primed)."""
            for h in range(H):
                frs = [(fi, f) for fi, f in enumerate(FRAGS) if f[0] == h]
                sc = psc.tile([S, TC], F32, tag="sc")
                for i, (fi, (hh, j, p0, p1)) in enumerate(frs):
                    nc.tensor.matmul(sc[0:S, 0:tw], kstage[fi][:],
                                     qs[j][0:128, 0:tw],
                                     start=(i == 0), stop=(i == len(frs) - 1))
                _softmax_tail(h, sc, tw, aps)

        def emit_scores_singles(qs, tw, aps):
            for (h, fi) in SINGLE:
                j = FRAGS[fi][1]
                sc = psc.tile([S, TC], F32, tag="sc")
                nc.tensor.matmul(sc[0:S, 0:tw], kstage[fi][:],
                                 qs[j][0:128, 0:tw], start=True, stop=True)
                _softmax_tail(h, sc, tw, aps)

        def emit_scores_packed(tw, aps):
            for (h, fiA, fiB, j, p0, p1) in STRADDLE:
                sc = psc.tile([S, TC], F32, tag="sc")
                nc.tensor.matmul(sc[0:S, 0:tw], kpk[h][:],
                                 qpk[h][0:128, 0:tw], start=True, stop=True)
                _softmax_tail(h, sc, tw, aps)

        def emit_av_tile(j, aps, tw, dve_at=False):
            av = pav.tile([128, TC], F32, tag="av")
            frs = [(fi, f) for fi, f in enumerate(FRAGS) if f[1] == j]
            for i, (fi, (h, jj, p0, p1)) in enumerate(frs):
                nc.tensor.matmul(av[0:128, 0:tw],
                                 vb[:, 128 * fi:128 * (fi + 1)],
                                 aps[h][0:S, 0:tw],
                                 start=(i == 0), stop=(i == len(frs) - 1))
            at = atpool.tile([128, TC], BF16, tag=f"at{j}", name=f"at{j}")
            if j < AT_ACT_N and not dve_at:
                nc.scalar.activation(at[0:128, 0:tw], av[0:128, 0:tw],
                                     AF.Copy)
            else:
                nc.vector.tensor_copy(at[0:128, 0:tw], av[0:128, 0:tw])
            return at

        def emit_oproj(attn, t0, tw, store_eng=None):
            obt = opool.tile([128, EJ, TC], BF16, tag="ob")
            for p in range(EJ):
                op = pop.tile([128, TC], F32, tag="op")
                for j in range(EJ):
                    nc.tensor.matmul(op[0:128, 0:tw],
                                     wo_t[:, j, 128 * p:128 * (p + 1)],
                                     attn[j][0:128, 0:tw],
                                     start=(j == 0), stop=(j == EJ - 1))
                nc.scalar.activation(obt[0:128, p, 0:tw], op[0:128, 0:tw],
                                     AF.Identity, bias=bot[:, p:p + 1])
            # store on the ACT HWDGE ring: frees the SP ring for x/qpk
            # prefetches and pairs naturally with the ob copies above
            (store_eng or nc.scalar).dma_start(otr[:, :, t0:t0 + tw],
                                               obt[0:128, 0:EJ, 0:tw])

        # ---- setup PE work: Vproj (needs yt+wv), Kproj (wk), Q(0) ----
        emit_vproj()
        emit_kproj()
        emit_kpk()
        qs = emit_qproj(xp0, CHUNKS[0][1])
        wo_t = loadw("wo", dr["wo"], EJ, E, eng=nc.scalar)

        attn_prev = None
        tprev = None
        xps = [xp0, xp1]
        for it in range(NT):
            t0, tw = CHUNKS[it]
            # prefetch x(i+2) (x0/x1 loaded during setup)
            if it + 2 < NT:
                nt0, ntw = CHUNKS[it + 2]
                xp_next = xpool.tile([128, EJ, TC], BF16, tag="xp")
                nc.sync.dma_start(xp_next[0:128, 0:EJ, 0:ntw],
                                  xtr[:, :, nt0:nt0 + ntw])
                xps.append(xp_next)

            aps = {}
            if it == 0:
                emit_scores_all(qs, tw, aps)
            else:
                emit_scores_singles(qs, tw, aps)
                emit_scores_packed(tw, aps)

            if it > 0:
                # near the narrow tail chunks the ACT SEQ is congested;
                # route those stores back to the SP ring
                emit_oproj(attn_prev, tprev[0], tprev[1],
                           store_eng=nc.sync if it >= NT - 2 else None)

            if it < NT - 1:
                # Q(i+1) before AV(i): together with Oproj(i-1) it covers
                # the softmax normalization chain latency of chunk i.
                qs = emit_qproj(xps[it + 1], CHUNKS[it + 1][1])
                emit_qpk(qs, CHUNKS[it + 1][1])
                attn = [emit_av_tile(j, aps, tw) for j in range(EJ)]
            else:
                # last chunk: j-outer Oproj interleaved with AV tiles,
                # staggered by one so the at-copy latency is hidden.
                ops = [pop.tile([128, TC], F32, tag="op", name=f"opl{p}")
                       for p in range(2)]
                ops += [pq.tile([128, TC], F32, tag="qp", name=f"opl{p + 2}")
                        for p in range(2)]
                ops += [psc.tile([128, TC], F32, tag="sc", name="opl4")]
                attn = []
                obt = opool.tile([128, EJ, TC], BF16, tag="ob")

                def last_o_group(j):
                    for p in range(EJ):
                        nc.tensor.matmul(ops[p][0:128, 0:tw],
                                         wo_t[:, j, 128 * p:128 * (p + 1)],
                                         attn[j][0:128, 0:tw],
                                         start=(j == 0), stop=(j == EJ - 1))

                for j in range(EJ):
                    # at-copies on DVE: it is idle in the last chunk (no
                    # Q(i+1) bias work) while ACT still drains exp's
                    attn.append(emit_av_tile(j, aps, tw, dve_at=True))
                    if j >= 1:
                        last_o_group(j - 1)
                # final accumulation group: emit each p's closing matmul,
                # its psum->sbuf bias copy, and the store as soon as ready
                for p in range(EJ):
                    nc.tensor.matmul(ops[p][0:128, 0:tw],
                                     wo_t[:, EJ - 1, 128 * p:128 * (p + 1)],
                                     attn[EJ - 1][0:128, 0:tw],
                                     start=False, stop=True)
                    if p % 2 == 1:
                        nc.scalar.activation(obt[0:128, p, 0:tw],
                                             ops[p][0:128, 0:tw],
                                             AF.Identity,
                                             bias=bot[:, p:p + 1])
                    else:
                        nc.vector.tensor_scalar_add(obt[0:128, p, 0:tw],
                                                    ops[p][0:128, 0:tw],
                                                    bot[:, p:p + 1])
                    if p == 2:
                        nc.scalar.dma_start(otr[:, 0:3, t0:t0 + tw],
                                            obt[0:128, 0:3, 0:tw])
                nc.sync.dma_start(otr[:, 3:EJ, t0:t0 + tw],
                                  obt[0:128, 3:EJ, 0:tw])
            attn_prev, tprev = attn, (t0, tw)


def build_program():
    nc = bacc.Bacc("TRN2", target_bir_lowering=False, debug=False,
                   num_devices=B)
    dr = {}

    def din(name, shape, dt):
        dr[name] = nc.dram_tensor(name, shape, dt, kind="ExternalInput")
        return dr[name]

    din("xt", [E, T], BF16)
    din("yt", [CR, S], BF16)
    din("wq", [E, E], BF16)
    din("wk", [CR, E], BF16)
    din("wv", [CR, E], BF16)
    din("wo", [E, E], BF16)
    din("consts", [128, 27], F32)
    din("consts2", [1, E + S], BF16)
    dr["ot"] = nc.dram_tensor("ot", [E, T], BF16, kind="ExternalOutput")

    with tile.TileContext(nc) as tc:
        _emit(nc, tc, {k: v[:] for k, v in dr.items()})
    nc.compile()
    return nc


def make_in_maps(x, y, Wq, bq, Wk, bk, Wv, bv, Wo, bo):
    import ml_dtypes
    BF = ml_dtypes.bfloat16

    def fb(a):
        return np.ascontiguousarray(np.asarray(a, np.float32).astype(BF))

    consts = np.zeros((128, 27), np.float32)
    consts[:, 0:EJ] = np.asarray(bq, np.float32).reshape(EJ, 128).T
    consts[:, EJ:2 * EJ] = np.asarray(bk, np.float32).reshape(EJ, 128).T
    consts[:, 2 * EJ:3 * EJ] = np.asarray(bo, np.float32).reshape(EJ, 128).T
    for fi, (h, j, p0, p1) in enumerate(FRAGS):
        consts[p0:p1, 3 * EJ + fi] = 1.0
    consts2 = np.zeros((1, E + S), np.float32)
    consts2[0, 0:E] = np.asarray(bv, np.float32)
    consts2[0, E:E + S] = 1.0

    shared = dict(
        wq=fb(Wq), wk=fb(Wk), wv=fb(Wv), wo=fb(Wo),
        consts=consts, consts2=fb(consts2),
    )
    x = np.asarray(x, np.float32)
    y = np.asarray(y, np.float32)
    in_maps = []
    for b in range(B):
        m = dict(shared)
        m["xt"] = fb(x[b].T)
        m["yt"] = fb(y[b].T)
        in_maps.append(m)
    return in_maps


def assemble_output(results):
    return np.stack(
        [np.asarray(results[b]["ot"]).astype(np.float32).T
         for b in range(B)], axis=0)


_PROG = None


def _prog():
    global _PROG
    if _PROG is None:
        _PROG = build_program()
    return _PROG


def kernel(x, y, Wq, bq, Wk, bk, Wv, bv, Wo, bo):
    nc = _prog()
    in_maps = make_in_maps(x, y, Wq, bq, Wk, bk, Wv, bv, Wo, bo)
    res = run_bass_kernel_spmd(nc, in_maps, core_ids=list(range(B)))
    return assemble_output(res.results)



# revision 9
# speedup vs baseline: 1.1129x; 1.1129x over previous
"""Cross-attention kernel for Trainium2 (8 NeuronCores, batch-parallel).

Reference computation (per batch element b):
    q = x @ Wq + bq            # [T, E]
    k = y @ Wk + bk            # [S, E]
    v = y @ Wv + bv            # [S, E]
    per head h (D=80): scores = q_h @ k_h.T / sqrt(D); A = softmax(scores)
    attn = concat_h(A @ v_h)   # [T, E]
    out = attn @ Wo + bo       # [T, E]

Sharding: batch (8) across the 8 cores, one batch element per core.

Design (vs the f32 baseline; cost-model time 210us -> 134us):
  - bf16 for every DMA'd tensor (x, y, weights, output) and the matmul
    streams; PSUM accumulation stays f32, softmax sums stay f32. Halves
    HBM traffic and SBUF pressure (rel tolerance is 2e-2, measured HW
    error ~4.4e-3).
  - softmax denominators via gpsimd partition_all_reduce +
    reciprocal_approx_fast + normalize multiplies alternating DVE/Pool:
    removes the baseline's 8 per-chunk sum-of-exp matmuls and its
    latency-heavy DMA+partition_broadcast chain.
  - heads whose 80 features straddle a 128-partition tile boundary
    (h1/h3/h4/h6) get their k and q repacked into dedicated 128-row
    tiles (SBUF-SBUF DMA, one chunk ahead), collapsing their two score
    fragments into ONE matmul: scores cost 8 MMs/chunk instead of 12.
  - one big DMA per chunk for x loads and output stores ([128, 5, tw]
    rearranged views) instead of 5 small ones (HWDGE fixed cost paid
    once); weight loads split by column block so Vproj/Kproj/Qproj
    start as soon as their slice lands; wv/wk/wo loads and the output
    stores issue from the ACT-engine HWDGE ring (qActDynamicHW) so the
    two physical descriptor-generation rings run in parallel.
  - software pipelining: per chunk the PE stream is scores(i) ->
    Oproj(i-1) -> Qproj(i+1) -> AV(i), so the softmax normalization
    chain of chunk i hides under ~10us of projection matmuls.
  - PE warmup matmuls during the initial weight-DMA fill so the
    p-state/HAM ramp completes before real work arrives; narrow first/
    last chunks shrink pipeline fill and drain; the last chunk
    interleaves its O-projection j-groups with the AV tiles.

On-chip layout is feature-major (x and the output are transposed on the
host): xt = x[b].T [E, T]; q' = Wq.T@X' [E, T]; per-head zero-padded
k/v staging tiles (80-dim heads vs 128-partition tiles).
"""

import numpy as np

import concourse.bass as bass
import concourse.bass_isa as bass_isa
import concourse.mybir as mybir
import concourse.tile as tile
from concourse import bacc
from concourse.bass_utils import run_bass_kernel_spmd

F32 = mybir.dt.float32
F32R = mybir.dt.float32r
BF16 = mybir.dt.bfloat16
F8 = mybir.dt.float8e4
DR = mybir.MatmulPerfMode.DoubleRow
AF = mybir.ActivationFunctionType

B, T, E, CR, H, D, S = 8, 4096, 640, 768, 8, 80, 77
# Qproj runs in fp8 e4m3 DoubleRow mode (2 k-tiles per matmul at 0.5
# cycles/row): x is scaled by 16 and Wq by 512 on the host (keeps both
# inside e4m3's normal range, TRN max 240), both zero-padded from 5 to
# 6 k-tiles of 128 so the contraction is 3 DoubleRow matmuls; the
# 2^-13 descale folds into the q bias-add. Softmax attenuates the fp8
# quantization error (logit std ~0.33): predicted rel err 1.2e-2.
XK = 6                   # padded x/wq k-tiles
XSCALE, WQSCALE = 16.0, 512.0
QDESCALE = 1.0 / (XSCALE * WQSCALE)
TC = 512
# narrow edge chunks: chunk 0 starts sooner (less DMA before the pipeline
# start) and the last chunk's O-projection + store can't overlap anything.
CHUNKS = ([(0, 256)] + [(256 + 512 * i, 512) for i in range(7)]
          + [(3840, 256)])
NT = len(CHUNKS)
EJ = E // 128            # 5 e-tiles
CJ = CR // 128           # 6 cross-dim tiles
SCALE = float(1.0 / np.sqrt(D))
NWARM = 77               # PE warmup matmuls during DMA fill (75 is the
                         # sim optimum but sits one MM from a 1.3us
                         # phase-alignment cliff; 77 keeps margin)
AT_ACT_N = 4             # at-copies j < AT_ACT_N go to ACT, rest to DVE


def _frags():
    fr = []
    for h in range(H):
        e0, e1 = D * h, D * h + D
        for j in range(e0 // 128, (e1 - 1) // 128 + 1):
            p0, p1 = max(0, e0 - 128 * j), min(128, e1 - 128 * j)
            fr.append((h, j, p0, p1))
    return fr


FRAGS = _frags()         # 12 fragments
NF = len(FRAGS)
# heads whose 80 features straddle a 128-partition tile boundary (two
# fragments); their scores run as ONE matmul against a repacked q tile
STRADDLE = []            # (h, fiA, fiB, j, p0, p1): fragA=(j,p0,128), fragB=(j+1,0,p1)
for h in range(H):
    frs = [(fi, f) for fi, f in enumerate(FRAGS) if f[0] == h]
    if len(frs) == 2:
        (fiA, (_, jA, p0A, _)), (fiB, (_, jB, _, p1B)) = frs
        STRADDLE.append((h, fiA, fiB, jA, p0A, p1B))
SINGLE = [(h, [fi for fi, f in enumerate(FRAGS) if f[0] == h][0])
          for h in range(H)
          if len([fi for fi, f in enumerate(FRAGS) if f[0] == h]) == 1]


def _emit(nc, tc, dr):
    import contextlib

    ctx = contextlib.ExitStack()
    with ctx:
        cpool = ctx.enter_context(tc.tile_pool(name="const", bufs=1))
        pq = ctx.enter_context(tc.tile_pool(name="pq", bufs=2, space="PSUM"))
        psc = ctx.enter_context(tc.tile_pool(name="psc", bufs=2, space="PSUM"))
        pav = ctx.enter_context(tc.tile_pool(name="pav", bufs=2, space="PSUM"))
        pop = ctx.enter_context(tc.tile_pool(name="pop", bufs=2, space="PSUM"))
        xpool = ctx.enter_context(tc.tile_pool(name="xpool", bufs=2))
        qpool = ctx.enter_context(tc.tile_pool(name="qpool", bufs=2))
        apool = ctx.enter_context(tc.tile_pool(name="apool", bufs=2))
        arpool = ctx.enter_context(tc.tile_pool(name="arpool", bufs=3))
        atpool = ctx.enter_context(tc.tile_pool(name="atpool", bufs=2))
        opool = ctx.enter_context(tc.tile_pool(name="opool", bufs=2))

        xtr = dr["xt"].rearrange("(b p) c -> p b c", p=128)   # [128, XK, T]
        otr = dr["ot"].rearrange("(b p) c -> p b c", p=128)

        # ---- PE warmup: keep the array busy through the DMA fill ----
        wtile = cpool.tile([128, 128], BF16, tag="warm", name="warm")
        nc.vector.memset(wtile[:], 0.0)
        wps = pop.tile([128, 128], F32, tag="op", name="warmps")
        for _ in range(NWARM):
            nc.tensor.matmul(wps[:], wtile[:], wtile[:], start=True, stop=True)

        # ---- weight/const loads (order = DMA queue order). The big
        # weights are split by column block so dependent PE groups can
        # start as soon as their slice lands.
        def loadw(name, src2, nblk, cols, c0=0, c1=None, dt=BF16,
                  eng=None):
            c1 = cols if c1 is None else c1
            t = cpool.tile([128, nblk, c1 - c0], dt, tag=name, name=name)
            (eng or nc.sync).dma_start(
                t[:], src2.rearrange("(b p) c -> p b c", p=128)[:, :, c0:c1])
            return t

        ytp_t = loadw("yt", dr["yt"], CJ, S)
        consts2 = cpool.tile([1, E + S], BF16, tag="consts2", name="consts2")
        nc.sync.dma_start(consts2[:], dr["consts2"])
        consts = cpool.tile([128, 27], F32, tag="consts", name="consts")
        nc.sync.dma_start(consts[:], dr["consts"])
        # wv via the ACT-engine HWDGE ring (qActDynamicHW on HW): its
        # descriptor generation runs parallel to the SP-ring loads
        wv_a = loadw("wva", dr["wv"], CJ, E, 0, 512, eng=nc.scalar)
        wv_b = loadw("wvb", dr["wv"], CJ, E, 512, E, eng=nc.scalar)
        wk_a = loadw("wka", dr["wk"], CJ, E, 0, 256, eng=nc.scalar)
        wk_b = loadw("wkb", dr["wk"], CJ, E, 256, E, eng=nc.scalar)
        wq_a = loadw("wqa", dr["wq"], XK, E, 0, 256, dt=F8)
        xp0 = xpool.tile([128, XK, TC], F8, tag="xp", name="xp0")
        nc.sync.dma_start(xp0[0:128, 0:XK, 0:CHUNKS[0][1]],
                          xtr[:, :, CHUNKS[0][0]:CHUNKS[0][0] + CHUNKS[0][1]])
        wq_b = loadw("wqb", dr["wq"], XK, E, 256, E, dt=F8)
        xp1 = xpool.tile([128, XK, TC], F8, tag="xp", name="xp1")
        nc.sync.dma_start(xp1[0:128, 0:XK, 0:CHUNKS[1][1]],
                          xtr[:, :, CHUNKS[1][0]:CHUNKS[1][0] + CHUNKS[1][1]])
        # qpk(0) DMAs, wo + remaining x chunks are issued further down

        def wq_pair(j, m):
            # [128, 2, 128] stationary slice for DoubleRow k-tile pair m
            return (wq_a[:, 2 * m:2 * m + 2, 128 * j:128 * (j + 1)] if j < 2
                    else wq_b[:, 2 * m:2 * m + 2, 128 * (j - 2):128 * (j - 1)])

        def wk_col(j, c):
            return (wk_a[:, c, 128 * j:128 * (j + 1)] if j < 2
                    else wk_b[:, c, 128 * (j - 2):128 * (j - 1)])

        bqt = consts[:, 0:EJ]
        bkt = consts[:, EJ:2 * EJ]
        bot = consts[:, 2 * EJ:3 * EJ]
        kmask = consts[:, 3 * EJ:3 * EJ + NF]
        bvr = consts2[:, 0:E]
        ones77 = consts2[:, E:E + S]

        # ---- V projection -> vb fragments (zero-padded) ----
        # vb holds, per fragment, a [S, 128] slab that is zero outside the
        # head's partition range; built by memset + free-dim-offset copies.
        vb = cpool.tile([S, NF * 128], BF16, tag="vb", name="vb")
        nc.gpsimd.memset(vb[:], 0.0)

        def emit_vproj():
            for (n0, n1), wv_t in (((0, 512), wv_a), ((512, E), wv_b)):
                vp = psc.tile([S, n1 - n0], F32, tag="sc")
                for c in range(CJ):
                    nc.tensor.matmul(vp[:], ytp_t[:, c, :],
                                     wv_t[:, c, :],
                                     start=(c == 0), stop=False)
                nc.tensor.matmul(vp[:], ones77[:], bvr[:, n0:n1],
                                 start=False, stop=True)
                for fi, (h, j, p0, p1) in enumerate(FRAGS):
                    c0 = 128 * j
                    if not (n0 <= c0 and c0 + 128 <= n1):
                        continue
                    # on ACT (idle during setup): keeps DVE clear for the
                    # kstage ops that gate Kproj's psum recycling
                    nc.scalar.activation(
                        vb[:, 128 * fi + p0:128 * fi + p1],
                        vp[:, c0 - n0 + p0:c0 - n0 + p1], AF.Copy)

        # ---- K projection -> zero-masked per-fragment staging tiles ----
        kstage = [cpool.tile([128, S], BF16, tag=f"ks{fi}", name=f"ks{fi}")
                  for fi in range(NF)]

        def emit_kproj():
            for j in range(EJ):
                # alternate psum pools (pav is idle during setup) so the
                # kstage DVE ops don't gate the bank recycling
                kpool = pq if j % 2 == 0 else pav
                kp = kpool.tile([128, S], F32, tag="qp" if j % 2 == 0
                                else "av")
                for c in range(CJ):
                    nc.tensor.matmul(kp[:], wk_col(j, c),
                                     ytp_t[:, c, :], start=(c == 0),
                                     stop=(c == CJ - 1))
                for fi, (h, jj, p0, p1) in enumerate(FRAGS):
                    if jj != j:
                        continue
                    nc.vector.tensor_scalar(kstage[fi][:], kp[:],
                                            bkt[:, j:j + 1],
                                            kmask[:, fi:fi + 1],
                                            mybir.AluOpType.add,
                                            mybir.AluOpType.mult)

        def emit_qproj(xp, tw, qpools=None):
            qs = []
            for j in range(EJ):
                pool, ptag = (qpools[j] if qpools
                              else (pq, "qp"))
                qp = pool.tile([128, TC], F32, tag=ptag, name=f"qp{j}")
                for m in range(XK // 2):
                    nc.tensor.matmul(qp[0:128, 0:tw],
                                     wq_pair(j, m),
                                     xp[0:128, 2 * m:2 * m + 2, 0:tw],
                                     start=(m == 0), stop=(m == XK // 2 - 1),
                                     perf_mode=DR)
                q = qpool.tile([128, TC], BF16, tag=f"q{j}", name=f"q{j}")
                nc.vector.tensor_scalar(q[0:128, 0:tw], qp[0:128, 0:tw],
                                        QDESCALE, bqt[:, j:j + 1],
                                        mybir.AluOpType.mult,
                                        mybir.AluOpType.add)
                qs.append(q)
            return qs

        # ---- packed k/q staging for straddling heads: their two score
        # fragments collapse to ONE matmul against a q tile whose rows
        # 0:80 are the head's features (repacked via SBUF-SBUF DMA).
        kpk = {}
        qpk = {}
        for (h, fiA, fiB, j, p0, p1) in STRADDLE:
            kpk[h] = cpool.tile([128, S], BF16, tag=f"kpk{h}", name=f"kpk{h}")
            qpk[h] = cpool.tile([128, TC], BF16, tag=f"qpk{h}",
                                name=f"qpk{h}")
            nc.gpsimd.memset(kpk[h][:], 0.0)
            nc.gpsimd.memset(qpk[h][:], 0.0)

        def emit_kpk():
            for (h, fiA, fiB, j, p0, p1) in STRADDLE:
                nc.sync.dma_start(kpk[h][0:128 - p0, :],
                                  kstage[fiA][p0:128, :])
                nc.sync.dma_start(kpk[h][128 - p0:D, :],
                                  kstage[fiB][0:p1, :])

        def emit_qpk(qs, tw):
            for (h, fiA, fiB, j, p0, p1) in STRADDLE:
                nc.sync.dma_start(qpk[h][0:128 - p0, 0:tw],
                                  qs[j][p0:128, 0:tw])
                nc.sync.dma_start(qpk[h][128 - p0:D, 0:tw],
                                  qs[j + 1][0:p1, 0:tw])

        def _softmax_tail(h, sc, tw, aps):
            """exp -> allreduce(sumexp) -> approx-recip -> normalize.
            The normalize multiplies alternate DVE/Pool to spread the
            elementwise load (DVE is near-saturated in steady state)."""
            a = apool.tile([S, TC], BF16, tag=f"a{h}", name=f"a{h}")
            nc.scalar.activation(a[0:S, 0:tw], sc[0:S, 0:tw], AF.Exp,
                                 scale=SCALE)
            ar = arpool.tile([S, TC], F32, tag="ar")
            nc.gpsimd.partition_all_reduce(
                ar[0:S, 0:tw], a[0:S, 0:tw], channels=S,
                reduce_op=bass_isa.ReduceOp.add)
            nc.vector.reciprocal_approx_fast(ar[0:S, 0:tw], ar[0:S, 0:tw])
            eng = nc.vector if h % 2 == 0 else nc.gpsimd
            eng.tensor_mul(a[0:S, 0:tw], a[0:S, 0:tw], ar[0:S, 0:tw])
            aps[h] = a

        def emit_scores_all(qs, tw, aps):
            """fragment-accumulation scores for all heads (chunk 0 only,
            before the packed q staging pipeline is primed)."""
            for h in range(H):
                frs = [(fi, f) for fi, f in enumerate(FRAGS) if f[0] == h]
                sc = psc.tile([S, TC], F32, tag="sc")
                for i, (fi, (hh, j, p0, p1)) in enumerate(frs):
                    nc.tensor.matmul(sc[0:S, 0:tw], kstage[fi][:],
                                     qs[j][0:128, 0:tw],
                                     start=(i == 0), stop=(i == len(frs) - 1))
                _softmax_tail(h, sc, tw, aps)

        def emit_scores_singles(qs, tw, aps):
            for (h, fi) in SINGLE:
                j = FRAGS[fi][1]
                sc = psc.tile([S, TC], F32, tag="sc")
                nc.tensor.matmul(sc[0:S, 0:tw], kstage[fi][:],
                                 qs[j][0:128, 0:tw], start=True, stop=True)
                _softmax_tail(h, sc, tw, aps)

        def emit_scores_packed(tw, aps):
            for (h, fiA, fiB, j, p0, p1) in STRADDLE:
                sc = psc.tile([S, TC], F32, tag="sc")
                nc.tensor.matmul(sc[0:S, 0:tw], kpk[h][:],
                                 qpk[h][0:128, 0:tw], start=True, stop=True)
                _softmax_tail(h, sc, tw, aps)

        def emit_av_tile(j, aps, tw, dve_at=False):
            av = pav.tile([128, TC], F32, tag="av")
            frs = [(fi, f) for fi, f in enumerate(FRAGS) if f[1] == j]
            for i, (fi, (h, jj, p0, p1)) in enumerate(frs):
                nc.tensor.matmul(av[0:128, 0:tw],
                                 vb[:, 128 * fi:128 * (fi + 1)],
                                 aps[h][0:S, 0:tw],
                                 start=(i == 0), stop=(i == len(frs) - 1))
            at = atpool.tile([128, TC], BF16, tag=f"at{j}", name=f"at{j}")
            if j < AT_ACT_N and not dve_at:
                nc.scalar.activation(at[0:128, 0:tw], av[0:128, 0:tw],
                                     AF.Copy)
            else:
                nc.vector.tensor_copy(at[0:128, 0:tw], av[0:128, 0:tw])
            return at

        def emit_oproj(attn, t0, tw, store_eng=None):
            obt = opool.tile([128, EJ, TC], BF16, tag="ob")
            for p in range(EJ):
                op = pop.tile([128, TC], F32, tag="op")
                for j in range(EJ):
                    nc.tensor.matmul(op[0:128, 0:tw],
                                     wo_t[:, j, 128 * p:128 * (p + 1)],
                                     attn[j][0:128, 0:tw],
                                     start=(j == 0), stop=(j == EJ - 1))
                nc.scalar.activation(obt[0:128, p, 0:tw], op[0:128, 0:tw],
                                     AF.Identity, bias=bot[:, p:p + 1])
            # store on the ACT HWDGE ring: frees the SP ring for x/qpk
            # prefetches and pairs naturally with the ob copies above
            (store_eng or nc.scalar).dma_start(otr[:, :, t0:t0 + tw],
                                               obt[0:128, 0:EJ, 0:tw])

        # ---- setup PE work: Vproj (needs yt+wv), Kproj (wk), Q(0) ----
        emit_vproj()
        emit_kproj()
        emit_kpk()
        qs = emit_qproj(xp0, CHUNKS[0][1])
        wo_t = loadw("wo", dr["wo"], EJ, E, eng=nc.scalar)

        attn_prev = None
        tprev = None
        xps = [xp0, xp1]
        for it in range(NT):
            t0, tw = CHUNKS[it]
            # prefetch x(i+2) (x0/x1 loaded during setup)
            if it + 2 < NT:
                nt0, ntw = CHUNKS[it + 2]
                xp_next = xpool.tile([128, XK, TC], F8, tag="xp")
                nc.sync.dma_start(xp_next[0:128, 0:XK, 0:ntw],
                                  xtr[:, :, nt0:nt0 + ntw])
                xps.append(xp_next)

            aps = {}
            if it == 0:
                emit_scores_all(qs, tw, aps)
            else:
                emit_scores_singles(qs, tw, aps)
                emit_scores_packed(tw, aps)

            if it > 0:
                # near the narrow tail chunks the ACT SEQ is congested;
                # route those stores back to the SP ring
                emit_oproj(attn_prev, tprev[0], tprev[1],
                           store_eng=nc.sync if it >= NT - 2 else None)

            if it < NT - 1:
                # Q(i+1) before AV(i): together with Oproj(i-1) it covers
                # the softmax normalization chain latency of chunk i.
                qs = emit_qproj(xps[it + 1], CHUNKS[it + 1][1])
                emit_qpk(qs, CHUNKS[it + 1][1])
                attn = [emit_av_tile(j, aps, tw) for j in range(EJ)]
            else:
                # last chunk: j-outer Oproj interleaved with AV tiles,
                # staggered by one so the at-copy latency is hidden.
                ops = [pop.tile([128, TC], F32, tag="op", name=f"opl{p}")
                       for p in range(2)]
                ops += [pq.tile([128, TC], F32, tag="qp", name=f"opl{p + 2}")
                        for p in range(2)]
                ops += [psc.tile([128, TC], F32, tag="sc", name="opl4")]
                attn = []
                obt = opool.tile([128, EJ, TC], BF16, tag="ob")

                def last_o_group(j):
                    for p in range(EJ):
                        nc.tensor.matmul(ops[p][0:128, 0:tw],
                                         wo_t[:, j, 128 * p:128 * (p + 1)],
                                         attn[j][0:128, 0:tw],
                                         start=(j == 0), stop=(j == EJ - 1))

                for j in range(EJ):
                    # at-copies on DVE: it is idle in the last chunk (no
                    # Q(i+1) bias work) while ACT still drains exp's
                    attn.append(emit_av_tile(j, aps, tw, dve_at=True))
                    if j >= 1:
                        last_o_group(j - 1)
                # final accumulation group: emit each p's closing matmul,
                # its psum->sbuf bias copy, and the store as soon as ready
                for p in range(EJ):
                    nc.tensor.matmul(ops[p][0:128, 0:tw],
                                     wo_t[:, EJ - 1, 128 * p:128 * (p + 1)],
                                     attn[EJ - 1][0:128, 0:tw],
                                     start=False, stop=True)
                    if p % 2 == 1:
                        nc.scalar.activation(obt[0:128, p, 0:tw],
                                             ops[p][0:128, 0:tw],
                                             AF.Identity,
                                             bias=bot[:, p:p + 1])
                    else:
                        nc.vector.tensor_scalar_add(obt[0:128, p, 0:tw],
                                                    ops[p][0:128, 0:tw],
                                                    bot[:, p:p + 1])
                    if p == 2:
                        nc.scalar.dma_start(otr[:, 0:3, t0:t0 + tw],
                                            obt[0:128, 0:3, 0:tw])
                nc.sync.dma_start(otr[:, 3:EJ, t0:t0 + tw],
                                  obt[0:128, 3:EJ, 0:tw])
            attn_prev, tprev = attn, (t0, tw)


def build_program():
    nc = bacc.Bacc("TRN2", target_bir_lowering=False, debug=False,
                   num_devices=B)
    dr = {}

    def din(name, shape, dt):
        dr[name] = nc.dram_tensor(name, shape, dt, kind="ExternalInput")
        return dr[name]

    din("xt", [XK * 128, T], F8)
    din("yt", [CR, S], BF16)
    din("wq", [XK * 128, E], F8)
    din("wk", [CR, E], BF16)
    din("wv", [CR, E], BF16)
    din("wo", [E, E], BF16)
    din("consts", [128, 27], F32)
    din("consts2", [1, E + S], BF16)
    dr["ot"] = nc.dram_tensor("ot", [E, T], BF16, kind="ExternalOutput")

    with tile.TileContext(nc) as tc:
        _emit(nc, tc, {k: v[:] for k, v in dr.items()})
    nc.compile()
    return nc


def make_in_maps(x, y, Wq, bq, Wk, bk, Wv, bv, Wo, bo):
    import ml_dtypes
    BF = ml_dtypes.bfloat16
    F8NP = ml_dtypes.float8_e4m3

    def fb(a):
        return np.ascontiguousarray(np.asarray(a, np.float32).astype(BF))

    def f8pad(a, scale):
        # scale into e4m3's sweet spot and zero-pad rows to XK*128 so the
        # DoubleRow contraction runs over 3 clean k-tile pairs
        out = np.zeros((XK * 128, a.shape[1]), F8NP)
        out[:a.shape[0]] = (np.asarray(a, np.float32) * scale).astype(F8NP)
        return out

    consts = np.zeros((128, 27), np.float32)
    consts[:, 0:EJ] = np.asarray(bq, np.float32).reshape(EJ, 128).T
    consts[:, EJ:2 * EJ] = np.asarray(bk, np.float32).reshape(EJ, 128).T
    consts[:, 2 * EJ:3 * EJ] = np.asarray(bo, np.float32).reshape(EJ, 128).T
    for fi, (h, j, p0, p1) in enumerate(FRAGS):
        consts[p0:p1, 3 * EJ + fi] = 1.0
    consts2 = np.zeros((1, E + S), np.float32)
    consts2[0, 0:E] = np.asarray(bv, np.float32)
    consts2[0, E:E + S] = 1.0

    shared = dict(
        wq=f8pad(Wq, WQSCALE), wk=fb(Wk), wv=fb(Wv), wo=fb(Wo),
        consts=consts, consts2=fb(consts2),
    )
    x = np.asarray(x, np.float32)
    y = np.asarray(y, np.float32)
    in_maps = []
    for b in range(B):
        m = dict(shared)
        m["xt"] = f8pad(x[b].T, XSCALE)
        m["yt"] = fb(y[b].T)
        in_maps.append(m)
    return in_maps


def assemble_output(results):
    return np.stack(
        [np.asarray(results[b]["ot"]).astype(np.float32).T
         for b in range(B)], axis=0)


_PROG = None


def _prog():
    global _PROG
    if _PROG is None:
        _PROG = build_program()
    return _PROG


def kernel(x, y, Wq, bq, Wk, bk, Wv, bv, Wo, bo):
    nc = _prog()
    in_maps = make_in_maps(x, y, Wq, bq, Wk, bk, Wv, bv, Wo, bo)
    res = run_bass_kernel_spmd(nc, in_maps, core_ids=list(range(B)))
    return assemble_output(res.results)



# revision 18
# speedup vs baseline: 1.2208x; 1.0970x over previous
"""Cross-attention kernel for Trainium2 (8 NeuronCores, batch-parallel).

Reference computation (per batch element b):
    q = x @ Wq + bq            # [T, E]
    k = y @ Wk + bk            # [S, E]
    v = y @ Wv + bv            # [S, E]
    per head h (D=80): scores = q_h @ k_h.T / sqrt(D); A = softmax(scores)
    attn = concat_h(A @ v_h)   # [T, E]
    out = attn @ Wo + bo       # [T, E]

Sharding: batch (8) across the 8 cores, one batch element per core.

Design (vs the f32 baseline; cost-model time 210us -> 134us):
  - bf16 for every DMA'd tensor (x, y, weights, output) and the matmul
    streams; PSUM accumulation stays f32, softmax sums stay f32. Halves
    HBM traffic and SBUF pressure (rel tolerance is 2e-2, measured HW
    error ~4.4e-3).
  - softmax denominators via gpsimd partition_all_reduce +
    reciprocal_approx_fast + normalize multiplies alternating DVE/Pool:
    removes the baseline's 8 per-chunk sum-of-exp matmuls and its
    latency-heavy DMA+partition_broadcast chain.
  - heads whose 80 features straddle a 128-partition tile boundary
    (h1/h3/h4/h6) get their k and q repacked into dedicated 128-row
    tiles (SBUF-SBUF DMA, one chunk ahead), collapsing their two score
    fragments into ONE matmul: scores cost 8 MMs/chunk instead of 12.
  - one big DMA per chunk for x loads and output stores ([128, 5, tw]
    rearranged views) instead of 5 small ones (HWDGE fixed cost paid
    once); weight loads split by column block so Vproj/Kproj/Qproj
    start as soon as their slice lands; wv/wk/wo loads and the output
    stores issue from the ACT-engine HWDGE ring (qActDynamicHW) so the
    two physical descriptor-generation rings run in parallel.
  - software pipelining: per chunk the PE stream is scores(i) ->
    Oproj(i-1) -> Qproj(i+1) -> AV(i), so the softmax normalization
    chain of chunk i hides under ~10us of projection matmuls.
  - PE warmup matmuls during the initial weight-DMA fill so the
    p-state/HAM ramp completes before real work arrives; narrow first/
    last chunks shrink pipeline fill and drain; the last chunk
    interleaves its O-projection j-groups with the AV tiles.

On-chip layout is feature-major (x and the output are transposed on the
host): xt = x[b].T [E, T]; q' = Wq.T@X' [E, T]; per-head zero-padded
k/v staging tiles (80-dim heads vs 128-partition tiles).
"""

import numpy as np

import concourse.bass as bass
import concourse.bass_isa as bass_isa
import concourse.mybir as mybir
import concourse.tile as tile
from concourse import bacc
from concourse.bass_utils import run_bass_kernel_spmd

F32 = mybir.dt.float32
F32R = mybir.dt.float32r
BF16 = mybir.dt.bfloat16
F8 = mybir.dt.float8e4
DR = mybir.MatmulPerfMode.DoubleRow
AF = mybir.ActivationFunctionType

B, T, E, CR, H, D, S = 8, 4096, 640, 768, 8, 80, 77
# Qproj runs in fp8 e4m3 DoubleRow mode (2 k-tiles per matmul at 0.5
# cycles/row): x is scaled by 16 and Wq by 512 on the host (keeps both
# inside e4m3's normal range, TRN max 240), both zero-padded from 5 to
# 6 k-tiles of 128 so the contraction is 3 DoubleRow matmuls; the
# 2^-13 descale folds into the q bias-add. Softmax attenuates the fp8
# quantization error (logit std ~0.33): predicted rel err 1.2e-2.
XK = 6                   # padded x/wq k-tiles
XSCALE, WQSCALE = 16.0, 512.0
QDESCALE = 1.0 / (XSCALE * WQSCALE)
TC = 512
# narrow edge chunks: chunk 0 starts sooner (less DMA before the pipeline
# start) and the last chunk's O-projection + store can't overlap anything.
CHUNKS = ([(0, 256)] + [(256 + 512 * i, 512) for i in range(7)]
          + [(3840, 256)])
NT = len(CHUNKS)
EJ = E // 128            # 5 e-tiles
CJ = CR // 128           # 6 cross-dim tiles
SCALE = float(1.0 / np.sqrt(D))
NWARM = 77               # PE warmup matmuls during DMA fill (75 is the
                         # sim optimum but sits one MM from a 1.3us
                         # phase-alignment cliff; 77 keeps margin)


def _frags():
    fr = []
    for h in range(H):
        e0, e1 = D * h, D * h + D
        for j in range(e0 // 128, (e1 - 1) // 128 + 1):
            p0, p1 = max(0, e0 - 128 * j), min(128, e1 - 128 * j)
            fr.append((h, j, p0, p1))
    return fr


FRAGS = _frags()         # 12 fragments
NF = len(FRAGS)
# heads whose 80 features straddle a 128-partition tile boundary (two
# fragments); their scores run as ONE matmul against a repacked q tile
STRADDLE = []            # (h, fiA, fiB, j, p0, p1): fragA=(j,p0,128), fragB=(j+1,0,p1)
for h in range(H):
    frs = [(fi, f) for fi, f in enumerate(FRAGS) if f[0] == h]
    if len(frs) == 2:
        (fiA, (_, jA, p0A, _)), (fiB, (_, jB, _, p1B)) = frs
        STRADDLE.append((h, fiA, fiB, jA, p0A, p1B))
SINGLE = [(h, [fi for fi, f in enumerate(FRAGS) if f[0] == h][0])
          for h in range(H)
          if len([fi for fi, f in enumerate(FRAGS) if f[0] == h]) == 1]


def _emit(nc, tc, dr):
    import contextlib

    ctx = contextlib.ExitStack()
    with ctx:
        cpool = ctx.enter_context(tc.tile_pool(name="const", bufs=1))
        pq = ctx.enter_context(tc.tile_pool(name="pq", bufs=2, space="PSUM"))
        psc = ctx.enter_context(tc.tile_pool(name="psc", bufs=2, space="PSUM"))
        pav = ctx.enter_context(tc.tile_pool(name="pav", bufs=2, space="PSUM"))
        pop = ctx.enter_context(tc.tile_pool(name="pop", bufs=2, space="PSUM"))
        xpool = ctx.enter_context(tc.tile_pool(name="xpool", bufs=2))
        qpool = ctx.enter_context(tc.tile_pool(name="qpool", bufs=2))
        apool = ctx.enter_context(tc.tile_pool(name="apool", bufs=2))
        arpool = ctx.enter_context(tc.tile_pool(name="arpool", bufs=3))
        atpool = ctx.enter_context(tc.tile_pool(name="atpool", bufs=2))
        opool = ctx.enter_context(tc.tile_pool(name="opool", bufs=2))

        xtr = dr["xt"].rearrange("(b p) c -> p b c", p=128)   # [128, XK, T]
        otr = dr["ot"].rearrange("(b p) c -> p b c", p=128)

        # ---- PE warmup: keep the array busy through the DMA fill ----
        wtile = cpool.tile([128, 128], BF16, tag="warm", name="warm")
        nc.vector.memset(wtile[:], 0.0)
        wps = pop.tile([128, 128], F32, tag="op", name="warmps")
        for _ in range(NWARM):
            nc.tensor.matmul(wps[:], wtile[:], wtile[:], start=True, stop=True)

        # ---- weight/const loads (order = DMA queue order). The big
        # weights are split by column block so dependent PE groups can
        # start as soon as their slice lands.
        def loadw(name, src2, nblk, cols, c0=0, c1=None, dt=BF16,
                  eng=None):
            c1 = cols if c1 is None else c1
            t = cpool.tile([128, nblk, c1 - c0], dt, tag=name, name=name)
            (eng or nc.sync).dma_start(
                t[:], src2.rearrange("(b p) c -> p b c", p=128)[:, :, c0:c1])
            return t

        ytp_t = loadw("yt", dr["yt"], CJ, S)
        consts2 = cpool.tile([1, E + S], BF16, tag="consts2", name="consts2")
        nc.sync.dma_start(consts2[:], dr["consts2"])
        consts = cpool.tile([128, 27], F32, tag="consts", name="consts")
        nc.sync.dma_start(consts[:], dr["consts"])
        # wv via the ACT-engine HWDGE ring (qActDynamicHW on HW): its
        # descriptor generation runs parallel to the SP-ring loads
        wv_a = loadw("wva", dr["wv"], CJ, E, 0, 512, eng=nc.scalar)
        wv_b = loadw("wvb", dr["wv"], CJ, E, 512, E, eng=nc.scalar)
        wk_a = loadw("wka", dr["wk"], CJ, E, 0, 256, eng=nc.scalar)
        wk_b = loadw("wkb", dr["wk"], CJ, E, 256, E, eng=nc.scalar)
        wq_a = loadw("wqa", dr["wq"], XK, E, 0, 256, dt=F8)
        xp0 = xpool.tile([128, XK, TC], F8, tag="xp", name="xp0")
        nc.sync.dma_start(xp0[0:128, 0:XK, 0:CHUNKS[0][1]],
                          xtr[:, :, CHUNKS[0][0]:CHUNKS[0][0] + CHUNKS[0][1]])
        wq_b = loadw("wqb", dr["wq"], XK, E, 256, E, dt=F8)
        xp1 = xpool.tile([128, XK, TC], F8, tag="xp", name="xp1")
        nc.sync.dma_start(xp1[0:128, 0:XK, 0:CHUNKS[1][1]],
                          xtr[:, :, CHUNKS[1][0]:CHUNKS[1][0] + CHUNKS[1][1]])
        # qpk(0) DMAs, wo + remaining x chunks are issued further down

        def wq_pair(j, m):
            # [128, 2, 128] stationary slice for DoubleRow k-tile pair m
            return (wq_a[:, 2 * m:2 * m + 2, 128 * j:128 * (j + 1)] if j < 2
                    else wq_b[:, 2 * m:2 * m + 2, 128 * (j - 2):128 * (j - 1)])

        def wk_col(j, c):
            return (wk_a[:, c, 128 * j:128 * (j + 1)] if j < 2
                    else wk_b[:, c, 128 * (j - 2):128 * (j - 1)])

        bqt = consts[:, 0:EJ]
        bkt = consts[:, EJ:2 * EJ]
        bot = consts[:, 2 * EJ:3 * EJ]
        kmask = consts[:, 3 * EJ:3 * EJ + NF]
        bvr = consts2[:, 0:E]
        ones77 = consts2[:, E:E + S]

        # ---- V projection -> vb fragments (zero-padded) ----
        # vb holds, per fragment, a [S, 128] slab that is zero outside the
        # head's partition range; built by memset + free-dim-offset copies.
        vb = cpool.tile([S, NF * 128], BF16, tag="vb", name="vb")
        nc.gpsimd.memset(vb[:], 0.0)

        def emit_vproj():
            for (n0, n1), wv_t in (((0, 512), wv_a), ((512, E), wv_b)):
                vp = psc.tile([S, n1 - n0], F32, tag="sc")
                for c in range(CJ):
                    nc.tensor.matmul(vp[:], ytp_t[:, c, :],
                                     wv_t[:, c, :],
                                     start=(c == 0), stop=False)
                nc.tensor.matmul(vp[:], ones77[:], bvr[:, n0:n1],
                                 start=False, stop=True)
                for fi, (h, j, p0, p1) in enumerate(FRAGS):
                    c0 = 128 * j
                    if not (n0 <= c0 and c0 + 128 <= n1):
                        continue
                    # on ACT (idle during setup): keeps DVE clear for the
                    # kstage ops that gate Kproj's psum recycling
                    nc.scalar.activation(
                        vb[:, 128 * fi + p0:128 * fi + p1],
                        vp[:, c0 - n0 + p0:c0 - n0 + p1], AF.Copy)

        # ---- K projection -> zero-masked per-fragment staging tiles ----
        kstage = [cpool.tile([128, S], BF16, tag=f"ks{fi}", name=f"ks{fi}")
                  for fi in range(NF)]

        def emit_kproj():
            for j in range(EJ):
                # alternate psum pools (pav is idle during setup) so the
                # kstage DVE ops don't gate the bank recycling
                kpool = pq if j % 2 == 0 else pav
                kp = kpool.tile([128, S], F32, tag="qp" if j % 2 == 0
                                else "av")
                for c in range(CJ):
                    nc.tensor.matmul(kp[:], wk_col(j, c),
                                     ytp_t[:, c, :], start=(c == 0),
                                     stop=(c == CJ - 1))
                for fi, (h, jj, p0, p1) in enumerate(FRAGS):
                    if jj != j:
                        continue
                    nc.vector.tensor_scalar(kstage[fi][:], kp[:],
                                            bkt[:, j:j + 1],
                                            kmask[:, fi:fi + 1],
                                            mybir.AluOpType.add,
                                            mybir.AluOpType.mult)

        def emit_qproj(xp, tw, qpools=None):
            qs = []
            for j in range(EJ):
                pool, ptag = (qpools[j] if qpools
                              else (pq, "qp"))
                qp = pool.tile([128, TC], F32, tag=ptag, name=f"qp{j}")
                for m in range(XK // 2):
                    nc.tensor.matmul(qp[0:128, 0:tw],
                                     wq_pair(j, m),
                                     xp[0:128, 2 * m:2 * m + 2, 0:tw],
                                     start=(m == 0), stop=(m == XK // 2 - 1),
                                     perf_mode=DR)
                q = qpool.tile([128, TC], BF16, tag=f"q{j}", name=f"q{j}")
                if j == 0:
                    # one q drain on ACT to balance the psum-drain load
                    nc.scalar.activation(q[0:128, 0:tw], qp[0:128, 0:tw],
                                         AF.Identity, bias=bqt[:, j:j + 1],
                                         scale=QDESCALE)
                else:
                    nc.vector.tensor_scalar(q[0:128, 0:tw], qp[0:128, 0:tw],
                                            QDESCALE, bqt[:, j:j + 1],
                                            mybir.AluOpType.mult,
                                            mybir.AluOpType.add)
                qs.append(q)
            return qs

        # ---- packed k/q staging for straddling heads: their two score
        # fragments collapse to ONE matmul against a q tile whose rows
        # 0:80 are the head's features (repacked via SBUF-SBUF DMA).
        kpk = {}
        qpk = {}
        for (h, fiA, fiB, j, p0, p1) in STRADDLE:
            kpk[h] = cpool.tile([128, S], BF16, tag=f"kpk{h}", name=f"kpk{h}")
            qpk[h] = cpool.tile([128, TC], BF16, tag=f"qpk{h}",
                                name=f"qpk{h}")
            nc.gpsimd.memset(kpk[h][:], 0.0)
            nc.gpsimd.memset(qpk[h][:], 0.0)

        def emit_kpk():
            for (h, fiA, fiB, j, p0, p1) in STRADDLE:
                nc.sync.dma_start(kpk[h][0:128 - p0, :],
                                  kstage[fiA][p0:128, :])
                nc.sync.dma_start(kpk[h][128 - p0:D, :],
                                  kstage[fiB][0:p1, :])

        def emit_qpk(qs, tw):
            for (h, fiA, fiB, j, p0, p1) in STRADDLE:
                nc.sync.dma_start(qpk[h][0:128 - p0, 0:tw],
                                  qs[j][p0:128, 0:tw])
                nc.sync.dma_start(qpk[h][128 - p0:D, 0:tw],
                                  qs[j + 1][0:p1, 0:tw])

        def _softmax_tail(h, sc, tw, aps):
            """exp -> allreduce(sumexp) -> approx-recip -> normalize.
            recip is DVE-only (fp32 bit trick; hw has no divide); the
            normalize multiplies all run on Pool, the cheapest engine
            for SBUF-SBUF elementwise work (427ns vs DVE 594)."""
            a = apool.tile([S, TC], BF16, tag=f"a{h}", name=f"a{h}")
            nc.scalar.activation(a[0:S, 0:tw], sc[0:S, 0:tw], AF.Exp,
                                 scale=SCALE)
            ar = arpool.tile([S, TC], F32, tag="ar")
            nc.gpsimd.partition_all_reduce(
                ar[0:S, 0:tw], a[0:S, 0:tw], channels=S,
                reduce_op=bass_isa.ReduceOp.add)
            nc.vector.reciprocal_approx_fast(ar[0:S, 0:tw], ar[0:S, 0:tw])
            nc.gpsimd.tensor_mul(a[0:S, 0:tw], a[0:S, 0:tw], ar[0:S, 0:tw])
            aps[h] = a

        def emit_scores_all(qs, tw, aps):
            """fragment-accumulation scores for all heads (chunk 0 only,
            before the packed q staging pipeline is primed)."""
            for h in range(H):
                frs = [(fi, f) for fi, f in enumerate(FRAGS) if f[0] == h]
                sc = psc.tile([S, TC], F32, tag="sc")
                for i, (fi, (hh, j, p0, p1)) in enumerate(frs):
                    nc.tensor.matmul(sc[0:S, 0:tw], kstage[fi][:],
                                     qs[j][0:128, 0:tw],
                                     start=(i == 0), stop=(i == len(frs) - 1))
                _softmax_tail(h, sc, tw, aps)

        def emit_scores_singles(qs, tw, aps):
            for (h, fi) in SINGLE:
                j = FRAGS[fi][1]
                sc = psc.tile([S, TC], F32, tag="sc")
                nc.tensor.matmul(sc[0:S, 0:tw], kstage[fi][:],
                                 qs[j][0:128, 0:tw], start=True, stop=True)
                _softmax_tail(h, sc, tw, aps)

        def emit_scores_packed(tw, aps):
            for (h, fiA, fiB, j, p0, p1) in STRADDLE:
                sc = psc.tile([S, TC], F32, tag="sc")
                nc.tensor.matmul(sc[0:S, 0:tw], kpk[h][:],
                                 qpk[h][0:128, 0:tw], start=True, stop=True)
                _softmax_tail(h, sc, tw, aps)

        def emit_av_tile(j, aps, tw, dve_at=False):
            av = pav.tile([128, TC], F32, tag="av")
            frs = [(fi, f) for fi, f in enumerate(FRAGS) if f[1] == j]
            for i, (fi, (h, jj, p0, p1)) in enumerate(frs):
                nc.tensor.matmul(av[0:128, 0:tw],
                                 vb[:, 128 * fi:128 * (fi + 1)],
                                 aps[h][0:S, 0:tw],
                                 start=(i == 0), stop=(i == len(frs) - 1))
            at = atpool.tile([128, TC], BF16, tag=f"at{j}", name=f"at{j}")
            # GPSIMD cannot access PSUM (BIR verifier), so the AV drains
            # split between ACT (j<2) and DVE
            if j < 2 and not dve_at:
                nc.scalar.activation(at[0:128, 0:tw], av[0:128, 0:tw],
                                     AF.Copy)
            else:
                nc.vector.tensor_copy(at[0:128, 0:tw], av[0:128, 0:tw])
            return at

        def emit_oproj(attn, t0, tw, store_eng=None):
            obt = opool.tile([128, EJ, TC], BF16, tag="ob")
            for p in range(EJ):
                op = pop.tile([128, TC], F32, tag="op")
                for j in range(EJ):
                    nc.tensor.matmul(op[0:128, 0:tw],
                                     wo_t[:, j, 128 * p:128 * (p + 1)],
                                     attn[j][0:128, 0:tw],
                                     start=(j == 0), stop=(j == EJ - 1))
                nc.scalar.activation(obt[0:128, p, 0:tw], op[0:128, 0:tw],
                                     AF.Identity, bias=bot[:, p:p + 1])
            # store on the SP ring: ACT's queue is the tighter budget now
            # that exp+obt copies live there (the DMA transfer time is
            # charged to the issuing engine's queue)
            (store_eng or nc.sync).dma_start(otr[:, :, t0:t0 + tw],
                                             obt[0:128, 0:EJ, 0:tw])

        # ---- setup PE work: Vproj (needs yt+wv), Kproj (wk), Q(0) ----
        emit_vproj()
        emit_kproj()
        emit_kpk()
        qs = emit_qproj(xp0, CHUNKS[0][1])
        wo_t = loadw("wo", dr["wo"], EJ, E, eng=nc.scalar)

        attn_prev = None
        tprev = None
        xps = [xp0, xp1]
        for it in range(NT):
            t0, tw = CHUNKS[it]
            # prefetch x(i+2) (x0/x1 loaded during setup)
            if it + 2 < NT:
                nt0, ntw = CHUNKS[it + 2]
                xp_next = xpool.tile([128, XK, TC], F8, tag="xp")
                nc.sync.dma_start(xp_next[0:128, 0:XK, 0:ntw],
                                  xtr[:, :, nt0:nt0 + ntw])
                xps.append(xp_next)

            aps = {}
            if it == 0:
                emit_scores_all(qs, tw, aps)
            else:
                emit_scores_singles(qs, tw, aps)
                emit_scores_packed(tw, aps)

            if it > 0:
                emit_oproj(attn_prev, tprev[0], tprev[1])

            if it < NT - 1:
                # Q(i+1) before AV(i): together with Oproj(i-1) it covers
                # the softmax normalization chain latency of chunk i.
                qs = emit_qproj(xps[it + 1], CHUNKS[it + 1][1])
                emit_qpk(qs, CHUNKS[it + 1][1])
                attn = [emit_av_tile(j, aps, tw) for j in range(EJ)]
            else:
                # last chunk: j-outer Oproj interleaved with AV tiles,
                # staggered by one so the at-copy latency is hidden.
                ops = [pop.tile([128, TC], F32, tag="op", name=f"opl{p}")
                       for p in range(2)]
                ops += [pq.tile([128, TC], F32, tag="qp", name=f"opl{p + 2}")
                        for p in range(2)]
                ops += [psc.tile([128, TC], F32, tag="sc", name="opl4")]
                attn = []
                obt = opool.tile([128, EJ, TC], BF16, tag="ob")

                def last_o_group(j):
                    for p in range(EJ):
                        nc.tensor.matmul(ops[p][0:128, 0:tw],
                                         wo_t[:, j, 128 * p:128 * (p + 1)],
                                         attn[j][0:128, 0:tw],
                                         start=(j == 0), stop=(j == EJ - 1))

                for j in range(EJ):
                    # at-copies on DVE: it is idle in the last chunk (no
                    # Q(i+1) bias work) while ACT still drains exp's
                    attn.append(emit_av_tile(j, aps, tw, dve_at=True))
                    if j >= 1:
                        last_o_group(j - 1)
                # final accumulation group: emit each p's closing matmul,
                # its psum->sbuf bias copy, and the store as soon as ready
                for p in range(EJ):
                    nc.tensor.matmul(ops[p][0:128, 0:tw],
                                     wo_t[:, EJ - 1, 128 * p:128 * (p + 1)],
                                     attn[EJ - 1][0:128, 0:tw],
                                     start=False, stop=True)
                    if p % 2 == 1:
                        nc.scalar.activation(obt[0:128, p, 0:tw],
                                             ops[p][0:128, 0:tw],
                                             AF.Identity,
                                             bias=bot[:, p:p + 1])
                    else:
                        nc.vector.tensor_scalar_add(obt[0:128, p, 0:tw],
                                                    ops[p][0:128, 0:tw],
                                                    bot[:, p:p + 1])
                    if p == 2:
                        nc.scalar.dma_start(otr[:, 0:3, t0:t0 + tw],
                                            obt[0:128, 0:3, 0:tw])
                nc.sync.dma_start(otr[:, 3:EJ, t0:t0 + tw],
                                  obt[0:128, 3:EJ, 0:tw])
            attn_prev, tprev = attn, (t0, tw)


def build_program():
    nc = bacc.Bacc("TRN2", target_bir_lowering=False, debug=False,
                   num_devices=B)
    dr = {}

    def din(name, shape, dt):
        dr[name] = nc.dram_tensor(name, shape, dt, kind="ExternalInput")
        return dr[name]

    din("xt", [XK * 128, T], F8)
    din("yt", [CR, S], BF16)
    din("wq", [XK * 128, E], F8)
    din("wk", [CR, E], BF16)
    din("wv", [CR, E], BF16)
    din("wo", [E, E], BF16)
    din("consts", [128, 27], F32)
    din("consts2", [1, E + S], BF16)
    dr["ot"] = nc.dram_tensor("ot", [E, T], BF16, kind="ExternalOutput")

    with tile.TileContext(nc) as tc:
        _emit(nc, tc, {k: v[:] for k, v in dr.items()})
    nc.compile()
    return nc


def make_in_maps(x, y, Wq, bq, Wk, bk, Wv, bv, Wo, bo):
    import ml_dtypes
    BF = ml_dtypes.bfloat16
    F8NP = ml_dtypes.float8_e4m3

    def fb(a):
        return np.ascontiguousarray(np.asarray(a, np.float32).astype(BF))

    def f8pad(a, scale):
        # scale into e4m3's sweet spot and zero-pad rows to XK*128 so the
        # DoubleRow contraction runs over 3 clean k-tile pairs
        out = np.zeros((XK * 128, a.shape[1]), F8NP)
        out[:a.shape[0]] = (np.asarray(a, np.float32) * scale).astype(F8NP)
        return out

    consts = np.zeros((128, 27), np.float32)
    consts[:, 0:EJ] = np.asarray(bq, np.float32).reshape(EJ, 128).T
    consts[:, EJ:2 * EJ] = np.asarray(bk, np.float32).reshape(EJ, 128).T
    consts[:, 2 * EJ:3 * EJ] = np.asarray(bo, np.float32).reshape(EJ, 128).T
    for fi, (h, j, p0, p1) in enumerate(FRAGS):
        consts[p0:p1, 3 * EJ + fi] = 1.0
    consts2 = np.zeros((1, E + S), np.float32)
    consts2[0, 0:E] = np.asarray(bv, np.float32)
    consts2[0, E:E + S] = 1.0

    shared = dict(
        wq=f8pad(Wq, WQSCALE), wk=fb(Wk), wv=fb(Wv), wo=fb(Wo),
        consts=consts, consts2=fb(consts2),
    )
    x = np.asarray(x, np.float32)
    y = np.asarray(y, np.float32)
    in_maps = []
    for b in range(B):
        m = dict(shared)
        m["xt"] = f8pad(x[b].T, XSCALE)
        m["yt"] = fb(y[b].T)
        in_maps.append(m)
    return in_maps


def assemble_output(results):
    return np.stack(
        [np.asarray(results[b]["ot"]).astype(np.float32).T
         for b in range(B)], axis=0)


_PROG = None


def _prog():
    global _PROG
    if _PROG is None:
        _PROG = build_program()
    return _PROG


def kernel(x, y, Wq, bq, Wk, bk, Wv, bv, Wo, bo):
    nc = _prog()
    in_maps = make_in_maps(x, y, Wq, bq, Wk, bk, Wv, bv, Wo, bo)
    res = run_bass_kernel_spmd(nc, in_maps, core_ids=list(range(B)))
    return assemble_output(res.results)



# revision 47
# speedup vs baseline: 1.3058x; 1.0696x over previous
"""Cross-attention kernel for Trainium2 (8 NeuronCores, batch-parallel).

Reference computation (per batch element b):
    q = x @ Wq + bq            # [T, E]
    k = y @ Wk + bk            # [S, E]
    v = y @ Wv + bv            # [S, E]
    per head h (D=80): scores = q_h @ k_h.T / sqrt(D); A = softmax(scores)
    attn = concat_h(A @ v_h)   # [T, E]
    out = attn @ Wo + bo       # [T, E]

Sharding: batch (8) across the 8 cores, one batch element per core.

Design (f32 baseline 210us -> bf16 134us -> fp8-Qproj 105us -> 98us):
  - THE Q PROJECTION IS GONE: scores_h = (Wq_h x + bq_h)^T k_h
    = x^T (Wq_h^T k_h) + (bq_h . k_h). Since k has only S=77 columns
    and is chunk-independent, k~_h = Wq_h^T k_h [E, S] is computed ONCE
    at setup (per head: 6 tiny matmuls against a host-transposed Wq)
    and quantized to fp8 e4m3 (x64). Scores then contract k~ directly
    with the fp8 x tiles over E in DoubleRow mode (3 paired matmuls,
    0.5 cy/row): 12 tw-units/chunk replace Qproj(7.5) + scores(8), and
    the 5 per-chunk q PSUM drains + the straddle-head q/k repacking
    DMAs disappear entirely. The q bias rides along for free: padded x
    row 640 is a constant 16 and wqt column 640 holds bq, so the
    k~ matmul emits (bq_h . k_h) into the c=640 row and the scores
    matmul adds it -- exp needs no bias operand.
  - error budget: x fp8 (2.7%) x k~ fp8 (2.7%) -> 3.8% logit error,
    attenuated ~x0.33 by softmax (logit std). numpy-sim 1.16e-2,
    measured 1.18e-2 vs the 2e-2 gate. fp8 CANNOT extend to AV/Oproj
    (A/V/attn quantization hits the output 1:1, ~3-4e-2).
  - engine rebalance around the drain-bandwidth wall (every PSUM byte
    must pass ACT or DVE exactly once; GPSIMD cannot touch PSUM, divide
    is not a hw TensorTensor op): exp + obt bias-drains on ACT;
    reciprocal (DVE-only fp32 bit trick) + AV drains on DVE; allreduce
    + ALL normalize multiplies on Pool (cheapest engine, 427ns flat);
    output stores on the SP ring (a dma_start charges its transfer to
    the issuing engine's queue).
  - weight loads split across all three HWDGE rings; Wk and Wq^T are
    head-major padded to 128 rows/cols per head on the host (partition
    strips only allow APs starting at 0/32/64/96, so an 80-row head
    slice at offset 80h is otherwise unaddressable).
  - per chunk the PE stream is scores(i) -> Oproj(i-1) -> AV(i); the
    softmax chains hide under Oproj. scores rotate through a 4-bank
    psc pool (freed by Qproj's removal), relaxing the exp-drain pacing.
  - PE warmup matmuls during the DMA fill so the p-state/HAM ramp
    completes before real work arrives; narrow first/last chunks shrink
    pipeline fill and drain; the last chunk interleaves its
    O-projection j-groups with the AV tiles.

On-chip layout is feature-major (x and the output are transposed on the
host): xt = x[b].T [768(pad), T] fp8; out ot [E, T] bf16; per-head
zero-padded v staging (80-dim heads vs 128-partition tiles).
"""

import numpy as np

import concourse.bass as bass
import concourse.bass_isa as bass_isa
import concourse.mybir as mybir
import concourse.tile as tile
from concourse import bacc
from concourse.bass_utils import run_bass_kernel_spmd

F32 = mybir.dt.float32
F32R = mybir.dt.float32r
BF16 = mybir.dt.bfloat16
F8 = mybir.dt.float8e4
DR = mybir.MatmulPerfMode.DoubleRow
AF = mybir.ActivationFunctionType

B, T, E, CR, H, D, S = 8, 4096, 640, 768, 8, 80, 77
XK = 6                   # padded x k-tiles (640 real + bias row 640)
XSCALE = 16.0            # x fp8 pre-scale (host)
KAP = 64.0               # k~ fp8 pre-scale (device drain)
TC = 512
# narrow edge chunks: chunk 0 starts sooner (less DMA before the pipeline
# start) and the last chunk's O-projection + store can't overlap anything.
CHUNKS = ([(0, 256)] + [(256 + 512 * i, 512) for i in range(7)]
          + [(3840, 256)])
NT = len(CHUNKS)
EJ = E // 128             # 5 e-tiles
CJ = CR // 128            # 6 cross-dim tiles
SCALE = float(1.0 / np.sqrt(D))
SSCALE = SCALE / (XSCALE * KAP)   # exp scale absorbs both fp8 pre-scales
NWARM = 30                # PE warmup matmuls during the DMA fill


def _frags():
    fr = []
    for h in range(H):
        e0, e1 = D * h, D * h + D
        for j in range(e0 // 128, (e1 - 1) // 128 + 1):
            p0, p1 = max(0, e0 - 128 * j), min(128, e1 - 128 * j)
            fr.append((h, j, p0, p1))
    return fr


FRAGS = _frags()          # 12 (head, j, p0, p1) fragments for Vproj/AV
NF = len(FRAGS)


def _emit(nc, tc, dr):
    import contextlib

    ctx = contextlib.ExitStack()
    with ctx:
        cpool = ctx.enter_context(tc.tile_pool(name="const", bufs=1))
        psc = ctx.enter_context(tc.tile_pool(name="psc", bufs=4, space="PSUM"))
        pav = ctx.enter_context(tc.tile_pool(name="pav", bufs=2, space="PSUM"))
        pop = ctx.enter_context(tc.tile_pool(name="pop", bufs=2, space="PSUM"))
        xpool = ctx.enter_context(tc.tile_pool(name="xpool", bufs=2))
        apool = ctx.enter_context(tc.tile_pool(name="apool", bufs=2))
        arpool = ctx.enter_context(tc.tile_pool(name="arpool", bufs=3))
        atpool = ctx.enter_context(tc.tile_pool(name="atpool", bufs=2))
        opool = ctx.enter_context(tc.tile_pool(name="opool", bufs=2))

        xtr = dr["xt"].rearrange("(b p) c -> p b c", p=128)   # [128, XK, T]
        otr = dr["ot"].rearrange("(b p) c -> p b c", p=128)

        # ---- PE warmup: keep the array busy through the DMA fill ----
        wtile = cpool.tile([128, 128], BF16, tag="warm", name="warm")
        nc.vector.memset(wtile[:], 0.0)
        wps = pop.tile([128, 128], F32, tag="op", name="warmps")
        for _ in range(NWARM):
            nc.tensor.matmul(wps[:], wtile[:], wtile[:], start=True, stop=True)

        # ---- loads (order = per-ring queue order) ----
        def loadw(name, src2, nblk, cols, c0=0, c1=None, dt=BF16,
                  eng=None):
            c1 = cols if c1 is None else c1
            t = cpool.tile([128, nblk, c1 - c0], dt, tag=name, name=name)
            (eng or nc.sync).dma_start(
                t[:], src2.rearrange("(b p) c -> p b c", p=128)[:, :, c0:c1])
            return t

        # SP ring: consts, yt (gates Kproj), wqt1, x chunks
        consts = cpool.tile([128, 13], F32, tag="consts", name="consts")
        nc.sync.dma_start(consts[:], dr["consts"])
        ytp_t = loadw("yt", dr["yt"], CJ, S)
        wqt1 = loadw("wqt1", dr["wqt1"], 4, CR)
        xp0 = xpool.tile([128, XK, TC], F8, tag="xp", name="xp0")
        nc.sync.dma_start(xp0[0:128, 0:XK, 0:CHUNKS[0][1]],
                          xtr[:, :, CHUNKS[0][0]:CHUNKS[0][0] + CHUNKS[0][1]])
        xp1 = xpool.tile([128, XK, TC], F8, tag="xp", name="xp1")
        nc.sync.dma_start(xp1[0:128, 0:XK, 0:CHUNKS[1][1]],
                          xtr[:, :, CHUNKS[1][0]:CHUNKS[1][0] + CHUNKS[1][1]])
        # ACT ring: wv halves + consts2 (Vproj bias)
        wv_a = loadw("wva", dr["wv"], CJ, E, 0, 512, eng=nc.scalar)
        consts2 = cpool.tile([1, E + S], BF16, tag="consts2", name="consts2")
        nc.scalar.dma_start(consts2[:], dr["consts2"])
        wv_b = loadw("wvb", dr["wv"], CJ, E, 512, E, eng=nc.scalar)
        # Pool ring: head-major-permuted Wk halves, wqt2, then wo (only
        # needed ~15us in; on ACT it blocked the k-hat/vb drain queue)
        wkp_a = loadw("wkpa", dr["wkp"], CJ, 1024, 0, 512, eng=nc.gpsimd)
        wkp_b = loadw("wkpb", dr["wkp"], CJ, 1024, 512, 1024, eng=nc.gpsimd)
        wqt2 = loadw("wqt2", dr["wqt2"], 4, CR, eng=nc.gpsimd)

        def wkp_col(h, c):
            return (wkp_a[:, c, 128 * h:128 * (h + 1)] if h < 4
                    else wkp_b[:, c, 128 * (h - 4):128 * (h - 3)])

        def wqt_col(h, cb):
            t = wqt1 if h < 4 else wqt2
            return t[:, h % 4, 128 * cb:128 * (cb + 1)]

        bot = consts[:, 0:EJ]
        bkp = consts[:, EJ:EJ + H]
        bvr = consts2[:, 0:E]
        ones77 = consts2[:, E:E + S]

        # ---- V projection -> vb fragments (zero-padded) ----
        vb = cpool.tile([S, NF * 128], BF16, tag="vb", name="vb")
        nc.gpsimd.memset(vb[:], 0.0)

        def emit_vproj():
            for (n0, n1), wv_t in (((0, 512), wv_a), ((512, E), wv_b)):
                vp = psc.tile([S, n1 - n0], F32, tag="sc")
                for c in range(CJ):
                    nc.tensor.matmul(vp[:], ytp_t[:, c, :],
                                     wv_t[:, c, :],
                                     start=(c == 0), stop=False)
                nc.tensor.matmul(vp[:], ones77[:], bvr[:, n0:n1],
                                 start=False, stop=True)
                for fi, (h, j, p0, p1) in enumerate(FRAGS):
                    c0 = 128 * j
                    if not (n0 <= c0 and c0 + 128 <= n1):
                        continue
                    nc.scalar.activation(
                        vb[:, 128 * fi + p0:128 * fi + p1],
                        vp[:, c0 - n0 + p0:c0 - n0 + p1], AF.Copy)

        # ---- K projection (head-major) + k~ = Wq_h^T k_h, fp8 x KAP.
        # kh8[:, h, :] = k_h [128 d-pad, S] bf16; kt8[:, h, cb, :] holds
        # the c-block cb of k~_h [128, S] fp8 (cb=5 row 0 = bq_h . k_h).
        kh8 = cpool.tile([128, H, S], BF16, tag="kh8", name="kh8")
        # k~ free dim padded 77->80: DoubleRow LDWEIGHTS requires the
        # pair-dim stride to be a multiple of 16 bytes. Pad columns are
        # zeroed once; the scores psum grows to 80 rows (77:80 unread).
        SP16 = 80
        kt8 = cpool.tile([128, H, XK, SP16], F8, tag="kt8", name="kt8")
        nc.vector.memset(kt8[:, :, :, S:SP16], 0.0)

        def emit_kproj(hs):
            # all heads back-to-back through the 6 psc/pav psum slots;
            # the k-hat drains (DVE/ACT alternating) pipeline behind
            for h in hs:
                kp = (psc if h % 2 == 0 else pav).tile(
                    [128, S], F32, tag="sc" if h % 2 == 0 else "av")
                for c in range(CJ):
                    nc.tensor.matmul(kp[:], wkp_col(h, c),
                                     ytp_t[:, c, :], start=(c == 0),
                                     stop=(c == CJ - 1))
                nc.vector.tensor_scalar_add(kh8[:, h, :], kp[:],
                                             bkp[:, h:h + 1])

        def emit_ktilde(hs):
            for h in hs:
                # 6 c-blocks into ONE psum bank (disjoint 77-col slices,
                # one start/stop accumulation chain)
                ktp = (pav if h % 2 == 0 else psc).tile(
                    [128, XK * S], F32, tag="av" if h % 2 == 0 else "sc")
                for cb in range(XK):
                    nc.tensor.matmul(ktp[0:128, S * cb:S * cb + S],
                                     wqt_col(h, cb), kh8[:, h, :],
                                     start=(cb == 0), stop=(cb == XK - 1),
                                     skip_group_check=True)
                if h % 2 == 0:
                    nc.scalar.activation(kt8[:, h, :, 0:S], ktp[:],
                                         AF.Identity, scale=KAP)
                else:
                    nc.vector.tensor_scalar_mul(kt8[:, h, :, 0:S], ktp[:],
                                                KAP)

        def _softmax_tail(h, sc, tw, aps):
            """exp -> allreduce(sumexp) -> approx-recip -> normalize.
            recip is DVE-only (fp32 bit trick; hw has no divide); the
            normalize multiplies all run on Pool."""
            a = apool.tile([S, TC], BF16, tag=f"a{h}", name=f"a{h}")
            nc.scalar.activation(a[0:S, 0:tw], sc[0:S, 0:tw], AF.Exp,
                                 scale=SSCALE)
            ar = arpool.tile([S, TC], F32, tag="ar")
            nc.gpsimd.partition_all_reduce(
                ar[0:S, 0:tw], a[0:S, 0:tw], channels=S,
                reduce_op=bass_isa.ReduceOp.add)
            nc.vector.reciprocal_approx_fast(ar[0:S, 0:tw], ar[0:S, 0:tw])
            nc.gpsimd.tensor_mul(a[0:S, 0:tw], a[0:S, 0:tw], ar[0:S, 0:tw])
            aps[h] = a

        def emit_scores(xp, tw, aps, hs=range(H)):
            # per head: 3 DoubleRow matmuls pairing x k-tiles (incl the
            # bias pair (4,5)); head order = AV consumption order
            for h in hs:
                sc = psc.tile([SP16, TC], F32, tag="sc")
                for m in range(XK // 2):
                    nc.tensor.matmul(sc[0:SP16, 0:tw],
                                     kt8[:, h, 2 * m:2 * m + 2, :],
                                     xp[0:128, 2 * m:2 * m + 2, 0:tw],
                                     start=(m == 0), stop=(m == XK // 2 - 1),
                                     perf_mode=DR)
                _softmax_tail(h, sc, tw, aps)

        def emit_av_tile(j, aps, tw):
            av = pav.tile([128, TC], F32, tag="av")
            frs = [(fi, f) for fi, f in enumerate(FRAGS) if f[1] == j]
            for i, (fi, (h, jj, p0, p1)) in enumerate(frs):
                nc.tensor.matmul(av[0:128, 0:tw],
                                 vb[:, 128 * fi:128 * (fi + 1)],
                                 aps[h][0:S, 0:tw],
                                 start=(i == 0), stop=(i == len(frs) - 1))
            at = atpool.tile([128, TC], BF16, tag=f"at{j}", name=f"at{j}")
            # GPSIMD cannot access PSUM; DVE has slack now that the q
            # drains are gone
            nc.vector.tensor_copy(at[0:128, 0:tw], av[0:128, 0:tw])
            return at

        def emit_oproj(attn, t0, tw, store_eng=None):
            obt = opool.tile([128, EJ, TC], BF16, tag="ob")
            for p in range(EJ):
                op = pop.tile([128, TC], F32, tag="op")
                for j in range(EJ):
                    nc.tensor.matmul(op[0:128, 0:tw],
                                     wo_t[:, j, 128 * p:128 * (p + 1)],
                                     attn[j][0:128, 0:tw],
                                     start=(j == 0), stop=(j == EJ - 1))
                nc.scalar.activation(obt[0:128, p, 0:tw], op[0:128, 0:tw],
                                     AF.Identity, bias=bot[:, p:p + 1])
            # store on the SP ring (the DMA transfer time is charged to
            # the issuing engine's queue; ACT is the tighter budget)
            (store_eng or nc.sync).dma_start(otr[:, :, t0:t0 + tw],
                                             obt[0:128, 0:EJ, 0:tw])

        # ---- setup PE work: Kproj all heads, Vproj (wv lands before
        # wqt1), then the k~ halves with chunk 0's scores interleaved so
        # the attention pipeline is already running when k~ completes.
        emit_kproj(range(0, H))
        emit_vproj()
        aps0 = {}
        emit_ktilde(range(0, 4))
        emit_scores(xp0, CHUNKS[0][1], aps0, range(0, 4))
        emit_ktilde(range(4, 8))
        emit_scores(xp0, CHUNKS[0][1], aps0, range(4, 8))
        wo_t = loadw("wo", dr["wo"], EJ, E, eng=nc.gpsimd)

        attn_prev = None
        tprev = None
        xps = [xp0, xp1]
        for it in range(NT):
            t0, tw = CHUNKS[it]
            # prefetch x(i+2) (x0/x1 loaded during setup)
            if it + 2 < NT:
                nt0, ntw = CHUNKS[it + 2]
                xp_next = xpool.tile([128, XK, TC], F8, tag="xp")
                nc.sync.dma_start(xp_next[0:128, 0:XK, 0:ntw],
                                  xtr[:, :, nt0:nt0 + ntw])
                xps.append(xp_next)

            if it == 0:
                aps = aps0
            else:
                aps = {}
                emit_scores(xps[it], tw, aps)
                emit_oproj(attn_prev, tprev[0], tprev[1])

            if it < NT - 1:
                attn = [emit_av_tile(j, aps, tw) for j in range(EJ)]
            else:
                # last chunk: j-outer Oproj interleaved with AV tiles,
                # staggered by one so the at-copy latency is hidden.
                ops = [pop.tile([128, TC], F32, tag="op", name=f"opl{p}")
                       for p in range(2)]
                ops += [psc.tile([128, TC], F32, tag="sc", name=f"opl{p + 2}")
                        for p in range(3)]
                attn = []
                obt = opool.tile([128, EJ, TC], BF16, tag="ob")

                def last_o_group(j):
                    for p in range(EJ):
                        nc.tensor.matmul(ops[p][0:128, 0:tw],
                                         wo_t[:, j, 128 * p:128 * (p + 1)],
                                         attn[j][0:128, 0:tw],
                                         start=(j == 0), stop=(j == EJ - 1))

                for j in range(EJ):
                    attn.append(emit_av_tile(j, aps, tw))
                    if j >= 1:
                        last_o_group(j - 1)
                # final accumulation group: emit each p's closing matmul,
                # its psum->sbuf bias copy, and the store as soon as ready
                for p in range(EJ):
                    nc.tensor.matmul(ops[p][0:128, 0:tw],
                                     wo_t[:, EJ - 1, 128 * p:128 * (p + 1)],
                                     attn[EJ - 1][0:128, 0:tw],
                                     start=False, stop=True)
                    if p % 2 == 1:
                        nc.scalar.activation(obt[0:128, p, 0:tw],
                                             ops[p][0:128, 0:tw],
                                             AF.Identity,
                                             bias=bot[:, p:p + 1])
                    else:
                        nc.vector.tensor_scalar_add(obt[0:128, p, 0:tw],
                                                    ops[p][0:128, 0:tw],
                                                    bot[:, p:p + 1])
                    if p == 2:
                        nc.scalar.dma_start(otr[:, 0:3, t0:t0 + tw],
                                            obt[0:128, 0:3, 0:tw])
                nc.sync.dma_start(otr[:, 3:EJ, t0:t0 + tw],
                                  obt[0:128, 3:EJ, 0:tw])
            attn_prev, tprev = attn, (t0, tw)


def build_program():
    nc = bacc.Bacc("TRN2", target_bir_lowering=False, debug=False,
                   num_devices=B)
    dr = {}

    def din(name, shape, dt):
        dr[name] = nc.dram_tensor(name, shape, dt, kind="ExternalInput")
        return dr[name]

    din("xt", [XK * 128, T], F8)
    din("yt", [CR, S], BF16)
    din("wqt1", [512, CR], BF16)
    din("wqt2", [512, CR], BF16)
    din("wkp", [CR, 1024], BF16)
    din("wv", [CR, E], BF16)
    din("wo", [E, E], BF16)
    din("consts", [128, 13], F32)
    din("consts2", [1, E + S], BF16)
    dr["ot"] = nc.dram_tensor("ot", [E, T], BF16, kind="ExternalOutput")

    with tile.TileContext(nc) as tc:
        _emit(nc, tc, {k: v[:] for k, v in dr.items()})
    nc.compile()
    return nc


def make_in_maps(x, y, Wq, bq, Wk, bk, Wv, bv, Wo, bo):
    import ml_dtypes
    BF = ml_dtypes.bfloat16
    F8NP = ml_dtypes.float8_e4m3

    def fb(a):
        return np.ascontiguousarray(np.asarray(a, np.float32).astype(BF))

    Wq = np.asarray(Wq, np.float32)
    bq = np.asarray(bq, np.float32)
    Wk = np.asarray(Wk, np.float32)
    bk = np.asarray(bk, np.float32)

    # wqt[128h+d, c] = Wq[c, 80h+d]; column 640 carries bq (the padded x
    # row 640 is the constant XSCALE, so scores pick up bq.k for free)
    wqt = np.zeros((1024, CR), np.float32)
    for h in range(H):
        wqt[128 * h:128 * h + D, 0:E] = Wq[:, D * h:D * h + D].T
        wqt[128 * h:128 * h + D, E] = bq[D * h:D * h + D]
    # wkp[:, 128h+d] = Wk[:, 80h+d]: head-major K projection output so
    # each head's 80 dims live in one partition-aligned 128-block
    wkp = np.zeros((CR, 1024), np.float32)
    for h in range(H):
        wkp[:, 128 * h:128 * h + D] = Wk[:, D * h:D * h + D]

    consts = np.zeros((128, 13), np.float32)
    consts[:, 0:EJ] = np.asarray(bo, np.float32).reshape(EJ, 128).T
    for h in range(H):
        consts[0:D, EJ + h] = bk[D * h:D * h + D]
    consts2 = np.zeros((1, E + S), np.float32)
    consts2[0, 0:E] = np.asarray(bv, np.float32)
    consts2[0, E:E + S] = 1.0

    shared = dict(
        wqt1=fb(wqt[:512]), wqt2=fb(wqt[512:]), wkp=fb(wkp),
        wv=fb(Wv), wo=fb(Wo), consts=consts, consts2=fb(consts2),
    )
    x = np.asarray(x, np.float32)
    y = np.asarray(y, np.float32)
    in_maps = []
    for b in range(B):
        m = dict(shared)
        xq = np.zeros((XK * 128, T), F8NP)
        xq[:E] = (x[b].T * XSCALE).astype(F8NP)
        xq[E] = F8NP(XSCALE)          # constant row: carries bq.k bias
        m["xt"] = xq
        m["yt"] = fb(y[b].T)
        in_maps.append(m)
    return in_maps


def assemble_output(results):
    return np.stack(
        [np.asarray(results[b]["ot"]).astype(np.float32).T
         for b in range(B)], axis=0)


_PROG = None


def _prog():
    global _PROG
    if _PROG is None:
        _PROG = build_program()
    return _PROG


def kernel(x, y, Wq, bq, Wk, bk, Wv, bv, Wo, bo):
    nc = _prog()
    in_maps = make_in_maps(x, y, Wq, bq, Wk, bk, Wv, bv, Wo, bo)
    res = run_bass_kernel_spmd(nc, in_maps, core_ids=list(range(B)))
    return assemble_output(res.results)


# revision 52
# speedup vs baseline: 1.3336x; 1.0213x over previous
"""Cross-attention kernel for Trainium2 (8 NeuronCores, batch-parallel).

Reference computation (per batch element b):
    q = x @ Wq + bq            # [T, E]
    k = y @ Wk + bk            # [S, E]
    v = y @ Wv + bv            # [S, E]
    per head h (D=80): scores = q_h @ k_h.T / sqrt(D); A = softmax(scores)
    attn = concat_h(A @ v_h)   # [T, E]
    out = attn @ Wo + bo       # [T, E]

Sharding: batch (8) across the 8 cores, one batch element per core.

Design (f32 baseline 210us -> bf16 134us -> fp8-Qproj 105us -> 100us):
  - THE Q PROJECTION IS GONE: scores_h = (Wq_h x + bq_h)^T k_h
    = x^T (Wq_h^T k_h) + (bq_h . k_h). Since k has only S=77 columns
    and is chunk-independent, k~_h = Wq_h^T k_h [E, S] is computed ONCE
    at setup (per head: 6 tiny matmuls against a host-transposed Wq)
    and quantized to fp8 e4m3 (x64). Scores then contract k~ directly
    with the fp8 x tiles over E in DoubleRow mode (3 paired matmuls,
    0.5 cy/row): 12 tw-units/chunk replace Qproj(7.5) + scores(8), and
    the 5 per-chunk q PSUM drains + the straddle-head q/k repacking
    DMAs disappear entirely. The q bias rides along for free: padded x
    row 640 is a constant 16 and wqt column 640 holds bq, so the
    k~ matmul emits (bq_h . k_h) into the c=640 row and the scores
    matmul adds it -- exp needs no bias operand.
  - error budget: x fp8 (2.7%) x k~ fp8 (2.7%) -> 3.8% logit error,
    attenuated ~x0.33 by softmax (logit std). numpy-sim 1.16e-2,
    measured 1.18e-2 vs the 2e-2 gate. fp8 CANNOT extend to AV/Oproj
    (A/V/attn quantization hits the output 1:1, ~3-4e-2).
  - engine rebalance around the drain-bandwidth wall (every PSUM byte
    must pass ACT or DVE exactly once; GPSIMD cannot touch PSUM, divide
    is not a hw TensorTensor op): exp + obt bias-drains on ACT;
    reciprocal (DVE-only fp32 bit trick) + AV drains on DVE; allreduce
    + ALL normalize multiplies on Pool (cheapest engine, 427ns flat);
    output stores on the SP ring (a dma_start charges its transfer to
    the issuing engine's queue).
  - weight loads split across all three HWDGE rings; Wk and Wq^T are
    head-major padded to 128 rows/cols per head on the host (partition
    strips only allow APs starting at 0/32/64/96, so an 80-row head
    slice at offset 80h is otherwise unaddressable). k~ tiles pad S
    77->80 (DoubleRow LDWEIGHTS needs pair-stride % 16 == 0). Vproj's
    bias is added in the vb drains against a host-broadcast [S, E] bv
    tile (bias varies along the free dim), deleting the ones-row bias
    matmul and its 1-partition const load. The fill interleaves
    Kproj h0-3 / Vproj / k~ h0-3 / Kproj h4-7 / chunk-0 scores h0-3 /
    k~ h4-7 / scores h4-7 so the PE tracks each DMA ring's landings.
  - per chunk the PE stream is scores(i) -> Oproj(i-1) -> AV(i); the
    softmax chains hide under Oproj. scores rotate through a 4-bank
    psc pool (freed by Qproj's removal), relaxing the exp-drain pacing.
  - PE warmup matmuls during the DMA fill so the p-state/HAM ramp
    completes before real work arrives; narrow first/last chunks shrink
    pipeline fill and drain; the last chunk interleaves its
    O-projection j-groups with the AV tiles.

On-chip layout is feature-major (x and the output are transposed on the
host): xt = x[b].T [768(pad), T] fp8; out ot [E, T] bf16; per-head
zero-padded v staging (80-dim heads vs 128-partition tiles).
"""

import numpy as np

import concourse.bass as bass
import concourse.bass_isa as bass_isa
import concourse.mybir as mybir
import concourse.tile as tile
from concourse import bacc
from concourse.bass_utils import run_bass_kernel_spmd

F32 = mybir.dt.float32
F32R = mybir.dt.float32r
BF16 = mybir.dt.bfloat16
F8 = mybir.dt.float8e4
DR = mybir.MatmulPerfMode.DoubleRow
AF = mybir.ActivationFunctionType

B, T, E, CR, H, D, S = 8, 4096, 640, 768, 8, 80, 77
XK = 6                   # padded x k-tiles (640 real + bias row 640)
XSCALE = 16.0            # x fp8 pre-scale (host)
KAP = 64.0               # k~ fp8 pre-scale (device drain)
TC = 512
# narrow edge chunks: chunk 0 starts sooner (less DMA before the pipeline
# start) and the last chunk's O-projection + store can't overlap anything.
CHUNKS = ([(0, 256)] + [(256 + 512 * i, 512) for i in range(7)]
          + [(3840, 256)])
NT = len(CHUNKS)
EJ = E // 128             # 5 e-tiles
CJ = CR // 128            # 6 cross-dim tiles
SCALE = float(1.0 / np.sqrt(D))
SSCALE = SCALE / (XSCALE * KAP)   # exp scale absorbs both fp8 pre-scales
NWARM = 14                # PE warmup matmuls during the DMA fill (the
                          # fill is load-gated; 10-18 all sim equal)


def _frags():
    fr = []
    for h in range(H):
        e0, e1 = D * h, D * h + D
        for j in range(e0 // 128, (e1 - 1) // 128 + 1):
            p0, p1 = max(0, e0 - 128 * j), min(128, e1 - 128 * j)
            fr.append((h, j, p0, p1))
    return fr


FRAGS = _frags()          # 12 (head, j, p0, p1) fragments for Vproj/AV
NF = len(FRAGS)


def _emit(nc, tc, dr):
    import contextlib

    ctx = contextlib.ExitStack()
    with ctx:
        cpool = ctx.enter_context(tc.tile_pool(name="const", bufs=1))
        psc = ctx.enter_context(tc.tile_pool(name="psc", bufs=4, space="PSUM"))
        pav = ctx.enter_context(tc.tile_pool(name="pav", bufs=2, space="PSUM"))
        pop = ctx.enter_context(tc.tile_pool(name="pop", bufs=2, space="PSUM"))
        xpool = ctx.enter_context(tc.tile_pool(name="xpool", bufs=2))
        apool = ctx.enter_context(tc.tile_pool(name="apool", bufs=2))
        arpool = ctx.enter_context(tc.tile_pool(name="arpool", bufs=3))
        atpool = ctx.enter_context(tc.tile_pool(name="atpool", bufs=2))
        opool = ctx.enter_context(tc.tile_pool(name="opool", bufs=2))

        xtr = dr["xt"].rearrange("(b p) c -> p b c", p=128)   # [128, XK, T]
        otr = dr["ot"].rearrange("(b p) c -> p b c", p=128)

        # ---- PE warmup: keep the array busy through the DMA fill ----
        wtile = cpool.tile([128, 128], BF16, tag="warm", name="warm")
        nc.vector.memset(wtile[:], 0.0)
        wps = pop.tile([128, 128], F32, tag="op", name="warmps")
        for _ in range(NWARM):
            nc.tensor.matmul(wps[:], wtile[:], wtile[:], start=True, stop=True)

        # ---- loads (order = per-ring queue order) ----
        def loadw(name, src2, nblk, cols, c0=0, c1=None, dt=BF16,
                  eng=None):
            c1 = cols if c1 is None else c1
            t = cpool.tile([128, nblk, c1 - c0], dt, tag=name, name=name)
            (eng or nc.sync).dma_start(
                t[:], src2.rearrange("(b p) c -> p b c", p=128)[:, :, c0:c1])
            return t

        # SP ring: consts, yt (gates Kproj), wqt1, x chunks
        consts = cpool.tile([128, 13], F32, tag="consts", name="consts")
        nc.sync.dma_start(consts[:], dr["consts"])
        ytp_t = loadw("yt", dr["yt"], CJ, S)
        wqt1 = loadw("wqt1", dr["wqt1"], 4, CR)
        xp0 = xpool.tile([128, XK, TC], F8, tag="xp", name="xp0")
        nc.sync.dma_start(xp0[0:128, 0:XK, 0:CHUNKS[0][1]],
                          xtr[:, :, CHUNKS[0][0]:CHUNKS[0][0] + CHUNKS[0][1]])
        xp1 = xpool.tile([128, XK, TC], F8, tag="xp", name="xp1")
        nc.sync.dma_start(xp1[0:128, 0:XK, 0:CHUNKS[1][1]],
                          xtr[:, :, CHUNKS[1][0]:CHUNKS[1][0] + CHUNKS[1][1]])
        # ACT ring: wv in three blocks (the act-table preamble delays
        # this ring 1.3us; smaller pieces land sooner) + broadcast bv
        wv_1 = loadw("wv1", dr["wv"], CJ, E, 0, 256, eng=nc.scalar)
        wv_2 = loadw("wv2", dr["wv"], CJ, E, 256, 512, eng=nc.scalar)
        wv_3 = loadw("wv3", dr["wv"], CJ, E, 512, E, eng=nc.scalar)
        bvb_t = cpool.tile([S, E], BF16, tag="bvb", name="bvb")
        nc.scalar.dma_start(bvb_t[:], dr["bvb"][:])
        # Pool ring: head-major-permuted Wk halves, wqt2, then wo (only
        # needed ~15us in; on ACT it blocked the k-hat/vb drain queue)
        wkp_a = loadw("wkpa", dr["wkp"], CJ, 1024, 0, 512, eng=nc.gpsimd)
        wkp_b = loadw("wkpb", dr["wkp"], CJ, 1024, 512, 1024, eng=nc.gpsimd)
        wqt2 = loadw("wqt2", dr["wqt2"], 4, CR, eng=nc.gpsimd)

        def wkp_col(h, c):
            return (wkp_a[:, c, 128 * h:128 * (h + 1)] if h < 4
                    else wkp_b[:, c, 128 * (h - 4):128 * (h - 3)])

        def wqt_col(h, cb):
            t = wqt1 if h < 4 else wqt2
            return t[:, h % 4, 128 * cb:128 * (cb + 1)]

        bot = consts[:, 0:EJ]
        bkp = consts[:, EJ:EJ + H]

        # ---- V projection -> vb fragments (zero-padded) ----
        vb = cpool.tile([S, NF * 128], BF16, tag="vb", name="vb")
        nc.gpsimd.memset(vb[:], 0.0)

        def emit_vproj():
            # bv added in the drain against a host-broadcast [S, E] tile
            # (the bias varies along the free dim, so no activation-bias
            # or ones-matmul needed); drains on DVE, idle in the fill
            for (n0, n1), wv_t in (((0, 256), wv_1), ((256, 512), wv_2),
                                   ((512, E), wv_3)):
                vp = psc.tile([S, n1 - n0], F32, tag="sc")
                for c in range(CJ):
                    nc.tensor.matmul(vp[:], ytp_t[:, c, :],
                                     wv_t[:, c, :],
                                     start=(c == 0), stop=(c == CJ - 1))
                for fi, (h, j, p0, p1) in enumerate(FRAGS):
                    c0 = 128 * j
                    if not (n0 <= c0 and c0 + 128 <= n1):
                        continue
                    nc.vector.tensor_tensor(
                        vb[:, 128 * fi + p0:128 * fi + p1],
                        vp[:, c0 - n0 + p0:c0 - n0 + p1],
                        bvb_t[:, c0 + p0:c0 + p1], mybir.AluOpType.add)

        # ---- K projection (head-major) + k~ = Wq_h^T k_h, fp8 x KAP.
        # kh8[:, h, :] = k_h [128 d-pad, S] bf16; kt8[:, h, cb, :] holds
        # the c-block cb of k~_h [128, S] fp8 (cb=5 row 0 = bq_h . k_h).
        kh8 = cpool.tile([128, H, S], BF16, tag="kh8", name="kh8")
        # k~ free dim padded 77->80: DoubleRow LDWEIGHTS requires the
        # pair-dim stride to be a multiple of 16 bytes. Pad columns are
        # zeroed once; the scores psum grows to 80 rows (77:80 unread).
        SP16 = 80
        kt8 = cpool.tile([128, H, XK, SP16], F8, tag="kt8", name="kt8")
        nc.vector.memset(kt8[:, :, :, S:SP16], 0.0)

        def emit_kproj(hs):
            # all heads back-to-back through the 6 psc/pav psum slots;
            # the k-hat drains (DVE/ACT alternating) pipeline behind
            for h in hs:
                kp = (psc if h % 2 == 0 else pav).tile(
                    [128, S], F32, tag="sc" if h % 2 == 0 else "av")
                for c in range(CJ):
                    nc.tensor.matmul(kp[:], wkp_col(h, c),
                                     ytp_t[:, c, :], start=(c == 0),
                                     stop=(c == CJ - 1))
                nc.vector.tensor_scalar_add(kh8[:, h, :], kp[:],
                                             bkp[:, h:h + 1])

        def emit_ktilde(hs):
            for h in hs:
                # 6 c-blocks into ONE psum bank (disjoint 77-col slices,
                # one start/stop accumulation chain)
                ktp = (pav if h % 2 == 0 else psc).tile(
                    [128, XK * S], F32, tag="av" if h % 2 == 0 else "sc")
                for cb in range(XK):
                    nc.tensor.matmul(ktp[0:128, S * cb:S * cb + S],
                                     wqt_col(h, cb), kh8[:, h, :],
                                     start=(cb == 0), stop=(cb == XK - 1),
                                     skip_group_check=True)
                if h % 2 == 0:
                    nc.scalar.activation(kt8[:, h, :, 0:S], ktp[:],
                                         AF.Identity, scale=KAP)
                else:
                    nc.vector.tensor_scalar_mul(kt8[:, h, :, 0:S], ktp[:],
                                                KAP)

        def _softmax_tail(h, sc, tw, aps):
            """exp -> allreduce(sumexp) -> approx-recip -> normalize.
            recip is DVE-only (fp32 bit trick; hw has no divide); the
            normalize multiplies all run on Pool."""
            a = apool.tile([S, TC], BF16, tag=f"a{h}", name=f"a{h}")
            nc.scalar.activation(a[0:S, 0:tw], sc[0:S, 0:tw], AF.Exp,
                                 scale=SSCALE)
            ar = arpool.tile([S, TC], F32, tag="ar")
            nc.gpsimd.partition_all_reduce(
                ar[0:S, 0:tw], a[0:S, 0:tw], channels=S,
                reduce_op=bass_isa.ReduceOp.add)
            nc.vector.reciprocal_approx_fast(ar[0:S, 0:tw], ar[0:S, 0:tw])
            nc.gpsimd.tensor_mul(a[0:S, 0:tw], a[0:S, 0:tw], ar[0:S, 0:tw])
            aps[h] = a

        def emit_scores(xp, tw, aps, hs=range(H)):
            # per head: 3 DoubleRow matmuls pairing x k-tiles (incl the
            # bias pair (4,5)); head order = AV consumption order
            for h in hs:
                sc = psc.tile([SP16, TC], F32, tag="sc")
                for m in range(XK // 2):
                    nc.tensor.matmul(sc[0:SP16, 0:tw],
                                     kt8[:, h, 2 * m:2 * m + 2, :],
                                     xp[0:128, 2 * m:2 * m + 2, 0:tw],
                                     start=(m == 0), stop=(m == XK // 2 - 1),
                                     perf_mode=DR)
                _softmax_tail(h, sc, tw, aps)

        def emit_av_tile(j, aps, tw):
            av = pav.tile([128, TC], F32, tag="av")
            frs = [(fi, f) for fi, f in enumerate(FRAGS) if f[1] == j]
            for i, (fi, (h, jj, p0, p1)) in enumerate(frs):
                nc.tensor.matmul(av[0:128, 0:tw],
                                 vb[:, 128 * fi:128 * (fi + 1)],
                                 aps[h][0:S, 0:tw],
                                 start=(i == 0), stop=(i == len(frs) - 1))
            at = atpool.tile([128, TC], BF16, tag=f"at{j}", name=f"at{j}")
            # GPSIMD cannot access PSUM; DVE has slack now that the q
            # drains are gone
            nc.vector.tensor_copy(at[0:128, 0:tw], av[0:128, 0:tw])
            return at

        def emit_oproj(attn, t0, tw, store_eng=None):
            obt = opool.tile([128, EJ, TC], BF16, tag="ob")
            for p in range(EJ):
                op = pop.tile([128, TC], F32, tag="op")
                for j in range(EJ):
                    nc.tensor.matmul(op[0:128, 0:tw],
                                     wo_t[:, j, 128 * p:128 * (p + 1)],
                                     attn[j][0:128, 0:tw],
                                     start=(j == 0), stop=(j == EJ - 1))
                nc.scalar.activation(obt[0:128, p, 0:tw], op[0:128, 0:tw],
                                     AF.Identity, bias=bot[:, p:p + 1])
            # store on the SP ring (the DMA transfer time is charged to
            # the issuing engine's queue; ACT is the tighter budget)
            (store_eng or nc.sync).dma_start(otr[:, :, t0:t0 + tw],
                                             obt[0:128, 0:EJ, 0:tw])

        # ---- setup PE work: Kproj all heads, Vproj (wv lands before
        # wqt1), then the k~ halves with chunk 0's scores interleaved so
        # the attention pipeline is already running when k~ completes.
        emit_kproj(range(0, 4))
        emit_vproj()          # wva lands ~4.3us, inside wkpb's shadow
        aps0 = {}
        emit_ktilde(range(0, 4))
        emit_kproj(range(4, 8))
        emit_scores(xp0, CHUNKS[0][1], aps0, range(0, 4))
        emit_ktilde(range(4, 8))
        emit_scores(xp0, CHUNKS[0][1], aps0, range(4, 8))
        wo_t = loadw("wo", dr["wo"], EJ, E, eng=nc.gpsimd)

        attn_prev = None
        tprev = None
        xps = [xp0, xp1]
        for it in range(NT):
            t0, tw = CHUNKS[it]
            # prefetch x(i+2) (x0/x1 loaded during setup)
            if it + 2 < NT:
                nt0, ntw = CHUNKS[it + 2]
                xp_next = xpool.tile([128, XK, TC], F8, tag="xp")
                nc.sync.dma_start(xp_next[0:128, 0:XK, 0:ntw],
                                  xtr[:, :, nt0:nt0 + ntw])
                xps.append(xp_next)

            if it == 0:
                aps = aps0
            else:
                aps = {}
                emit_scores(xps[it], tw, aps)
                emit_oproj(attn_prev, tprev[0], tprev[1])

            if it < NT - 1:
                attn = [emit_av_tile(j, aps, tw) for j in range(EJ)]
            else:
                # last chunk: j-outer Oproj interleaved with AV tiles,
                # staggered by one so the at-copy latency is hidden.
                ops = [pop.tile([128, TC], F32, tag="op", name=f"opl{p}")
                       for p in range(2)]
                ops += [psc.tile([128, TC], F32, tag="sc", name=f"opl{p + 2}")
                        for p in range(3)]
                attn = []
                obt = opool.tile([128, EJ, TC], BF16, tag="ob")

                def last_o_group(j):
                    for p in range(EJ):
                        nc.tensor.matmul(ops[p][0:128, 0:tw],
                                         wo_t[:, j, 128 * p:128 * (p + 1)],
                                         attn[j][0:128, 0:tw],
                                         start=(j == 0), stop=(j == EJ - 1))

                for j in range(EJ):
                    attn.append(emit_av_tile(j, aps, tw))
                    if j >= 1:
                        last_o_group(j - 1)
                # final accumulation group: emit each p's closing matmul,
                # its psum->sbuf bias copy, and the store as soon as ready
                for p in range(EJ):
                    nc.tensor.matmul(ops[p][0:128, 0:tw],
                                     wo_t[:, EJ - 1, 128 * p:128 * (p + 1)],
                                     attn[EJ - 1][0:128, 0:tw],
                                     start=False, stop=True)
                    if p % 2 == 1:
                        nc.scalar.activation(obt[0:128, p, 0:tw],
                                             ops[p][0:128, 0:tw],
                                             AF.Identity,
                                             bias=bot[:, p:p + 1])
                    else:
                        nc.vector.tensor_scalar_add(obt[0:128, p, 0:tw],
                                                    ops[p][0:128, 0:tw],
                                                    bot[:, p:p + 1])
                    if p == 2:
                        nc.scalar.dma_start(otr[:, 0:3, t0:t0 + tw],
                                            obt[0:128, 0:3, 0:tw])
                nc.sync.dma_start(otr[:, 3:EJ, t0:t0 + tw],
                                  obt[0:128, 3:EJ, 0:tw])
            attn_prev, tprev = attn, (t0, tw)


def build_program():
    nc = bacc.Bacc("TRN2", target_bir_lowering=False, debug=False,
                   num_devices=B)
    dr = {}

    def din(name, shape, dt):
        dr[name] = nc.dram_tensor(name, shape, dt, kind="ExternalInput")
        return dr[name]

    din("xt", [XK * 128, T], F8)
    din("yt", [CR, S], BF16)
    din("wqt1", [512, CR], BF16)
    din("wqt2", [512, CR], BF16)
    din("wkp", [CR, 1024], BF16)
    din("wv", [CR, E], BF16)
    din("wo", [E, E], BF16)
    din("consts", [128, 13], F32)
    din("bvb", [S, E], BF16)
    dr["ot"] = nc.dram_tensor("ot", [E, T], BF16, kind="ExternalOutput")

    with tile.TileContext(nc) as tc:
        _emit(nc, tc, {k: v[:] for k, v in dr.items()})
    nc.compile()
    return nc


def make_in_maps(x, y, Wq, bq, Wk, bk, Wv, bv, Wo, bo):
    import ml_dtypes
    BF = ml_dtypes.bfloat16
    F8NP = ml_dtypes.float8_e4m3

    def fb(a):
        return np.ascontiguousarray(np.asarray(a, np.float32).astype(BF))

    Wq = np.asarray(Wq, np.float32)
    bq = np.asarray(bq, np.float32)
    Wk = np.asarray(Wk, np.float32)
    bk = np.asarray(bk, np.float32)

    # wqt[128h+d, c] = Wq[c, 80h+d]; column 640 carries bq (the padded x
    # row 640 is the constant XSCALE, so scores pick up bq.k for free)
    wqt = np.zeros((1024, CR), np.float32)
    for h in range(H):
        wqt[128 * h:128 * h + D, 0:E] = Wq[:, D * h:D * h + D].T
        wqt[128 * h:128 * h + D, E] = bq[D * h:D * h + D]
    # wkp[:, 128h+d] = Wk[:, 80h+d]: head-major K projection output so
    # each head's 80 dims live in one partition-aligned 128-block
    wkp = np.zeros((CR, 1024), np.float32)
    for h in range(H):
        wkp[:, 128 * h:128 * h + D] = Wk[:, D * h:D * h + D]

    consts = np.zeros((128, 13), np.float32)
    consts[:, 0:EJ] = np.asarray(bo, np.float32).reshape(EJ, 128).T
    for h in range(H):
        consts[0:D, EJ + h] = bk[D * h:D * h + D]
    bvb = np.tile(np.asarray(bv, np.float32)[None, :], (S, 1))

    shared = dict(
        wqt1=fb(wqt[:512]), wqt2=fb(wqt[512:]), wkp=fb(wkp),
        wv=fb(Wv), wo=fb(Wo), consts=consts, bvb=fb(bvb),
    )
    x = np.asarray(x, np.float32)
    y = np.asarray(y, np.float32)
    in_maps = []
    for b in range(B):
        m = dict(shared)
        xq = np.zeros((XK * 128, T), F8NP)
        xq[:E] = (x[b].T * XSCALE).astype(F8NP)
        xq[E] = F8NP(XSCALE)          # constant row: carries bq.k bias
        m["xt"] = xq
        m["yt"] = fb(y[b].T)
        in_maps.append(m)
    return in_maps


def assemble_output(results):
    return np.stack(
        [np.asarray(results[b]["ot"]).astype(np.float32).T
         for b in range(B)], axis=0)


_PROG = None


def _prog():
    global _PROG
    if _PROG is None:
        _PROG = build_program()
    return _PROG


def kernel(x, y, Wq, bq, Wk, bk, Wv, bv, Wo, bo):
    nc = _prog()
    in_maps = make_in_maps(x, y, Wq, bq, Wk, bk, Wv, bv, Wo, bo)
    res = run_bass_kernel_spmd(nc, in_maps, core_ids=list(range(B)))
    return assemble_output(res.results)
